# revision 13
# baseline (speedup 1.0000x reference)
"""2-layer GAT (GATConv x2 + link predictor) on 8 Trainium2 NeuronCores.

Sharding: nodes partitioned into 8 contiguous blocks (graph parallel).
Each core aggregates incoming edges of its own destination block.
L1 source features are host-pre-gathered into edge-slot order (x is an
input); L2 aggregates the device-computed g-table (AllGather'd across
cores) with device-side indirect-DMA gathers, and link predictions
gather rows of the AllGather'd z-table.

Segment softmax / segment sum are done with per-tile mask matmuls on the
tensor engine: edges are sorted by destination and padded per 128-node
destination tile; M[e,d] = (dstloc[e]==d) maps 128 edges onto the tile's
nodes, so M.T @ (alpha * feat) accumulates per-node sums in PSUM.
"""
import numpy as np
import ml_dtypes

BF16 = ml_dtypes.bfloat16

# model dims (from the reference problem; fixed by the harness)
N_NODES = 50000
DIM = 128
HEADS = 4
HID = 256
EMB = 128
SLOPE = 0.2

NC = 8
P = 128
BLK = N_NODES // NC            # 6250 real nodes per core
NB = (BLK + P - 1) // P        # 49 dst tiles per core
BLKP = NB * P                  # 6272 padded rows per core
GW = 131                       # g-table row: [g(128) | 1.0 | as2 | ad2]
PAD_DST = 999.0


def _prep(x, e, p, n, W1, a_src1, a_dst1, b1, W2, a_src2, a_dst2, b2, cls_W, cls_b):
    """Host-side graph/index/layout prep. No model FLOPs on node data."""
    x = np.asarray(x, np.float32)
    e = np.asarray(e, np.int64)
    p = np.asarray(p, np.int64)
    n = np.asarray(n, np.int64)

    loop = np.arange(N_NODES, dtype=np.int64)
    src = np.concatenate([e[0], loop])
    dst = np.concatenate([e[1], loop])

    core_of = dst // BLK
    row_of = lambda g: (g // BLK) * BLKP + (g % BLK)  # node -> padded table row

    # per (core, dst-tile) edge lists
    tile_of = (dst % BLK) // P
    order = np.lexsort((dst, tile_of, core_of))
    src_s, dst_s = src[order], dst[order]
    core_s, tile_s = core_of[order], tile_of[order]
    key = core_s * NB + tile_s
    counts = np.bincount(key, minlength=NC * NB)
    t_fix = int(np.max((counts + P - 1) // P))
    nT = NB * t_fix
    n_slots = nT * P

    starts = np.zeros(NC * NB + 1, dtype=np.int64)
    np.cumsum(counts, out=starts[1:])

    xe = np.zeros((NC, n_slots, DIM), np.float32)
    dstloc_sl = np.full((NC, n_slots), PAD_DST, np.float32)
    srcg_sl = np.zeros((NC, n_slots), np.int32)
    for c in range(NC):
        for b in range(NB):
            k = c * NB + b
            cnt = counts[k]
            if cnt == 0:
                continue
            s0 = starts[k]
            base = b * t_fix * P
            sl = slice(base, base + cnt)
            es = src_s[s0:s0 + cnt]
            xe[c, sl] = x[es]
            dstloc_sl[c, sl] = (dst_s[s0:s0 + cnt] % BLK) % P
            srcg_sl[c, sl] = row_of(es)
    # slot s lives at [partition s%128, tile s//128]
    dstloc = dstloc_sl.reshape(NC, nT, P).transpose(0, 2, 1).copy()   # [NC,P,nT]
    srcg = srcg_sl.reshape(NC, nT, P).transpose(0, 2, 1).copy()
    xeT = np.ascontiguousarray(xe.transpose(0, 2, 1))                 # [NC,DIM,n_slots]
    # transposed masks MT[d, e] per tile, laid out [P, nT*P]
    dl = dstloc_sl.reshape(NC, nT, P)                                 # [c, t, e]
    mtb = (dl[:, :, None, :] == np.arange(P, dtype=np.float32)[None, None, :, None])
    mtb = np.ascontiguousarray(
        mtb.astype(np.float32).transpose(0, 2, 1, 3).reshape(NC, P, nT * P))

    # own-block x transposed, padded
    xTo = np.zeros((NC, DIM, BLKP), np.float32)
    for c in range(NC):
        xTo[c, :, :BLK] = x[c * BLK:(c + 1) * BLK].T

    # pred edges sharded by position
    n_pred = p.shape[1] + n.shape[1]
    pp = n_pred // NC
    npt = (pp + P - 1) // P
    pa = np.zeros((NC, npt * P), np.int32)
    pb = np.zeros((NC, npt * P), np.int32)
    allp = np.concatenate([p, n], axis=1)
    for c in range(NC):
        seg = allp[:, c * pp:(c + 1) * pp]
        pa[c, :pp] = row_of(seg[0])
        pb[c, :pp] = row_of(seg[1])
    pa = pa.reshape(NC, npt, P).transpose(0, 2, 1).copy()
    pb = pb.reshape(NC, npt, P).transpose(0, 2, 1).copy()

    # weight prep (weights only)
    W1 = np.asarray(W1, np.float32)
    W2 = np.asarray(W2, np.float32)
    W1h = W1.reshape(HEADS, HID, DIM)
    was1 = np.einsum('kh,khd->kd', np.asarray(a_src1, np.float32), W1h)
    wad1 = np.einsum('kh,khd->kd', np.asarray(a_dst1, np.float32), W1h)
    W1T = np.ascontiguousarray(W1.T)                                   # [128, 1024]
    b1 = np.asarray(b1, np.float32)
    b1T = b1.reshape(8, P).T.copy()                                    # [128, 8] col c
    W2T = np.ascontiguousarray(W2.T)                                   # [1024, 128]
    was2 = W2T @ np.asarray(a_src2, np.float32)[0]                     # [1024]
    wad2 = W2T @ np.asarray(a_dst2, np.float32)[0]
    W2aug = np.zeros((8, P, 130), np.float32)
    for c in range(8):
        W2aug[c, :, :128] = W2T[c * P:(c + 1) * P]
        W2aug[c, :, 128] = was2[c * P:(c + 1) * P]
        W2aug[c, :, 129] = wad2[c * P:(c + 1) * P]
    W2augP = np.ascontiguousarray(W2aug.transpose(1, 0, 2)).reshape(P, 8 * 130)
    b2_bc = np.tile(np.asarray(b2, np.float32)[None, :], (P, 1))       # [128, 128]
    clsWT = np.ascontiguousarray(np.asarray(cls_W, np.float32).T)      # [128, 4]
    clsb_bc = np.tile(np.asarray(cls_b, np.float32)[None, :], (P, 1))  # [128, 4]
    iota_f = np.tile(np.arange(P, dtype=np.float32)[None, :], (P, 1))
    ident = np.eye(P, dtype=np.float32)

    shared = dict(W1T=W1T, wasT1=np.ascontiguousarray(was1.T),
                  wadT1=np.ascontiguousarray(wad1.T), b1T=b1T,
                  W2augP=W2augP, b2_bc=b2_bc, clsWT=clsWT, clsb_bc=clsb_bc,
                  iota_f=iota_f, ident=ident)
    per_core = []
    for c in range(NC):
        m = dict(shared)
        m.update(xe=xe[c], xeT=xeT[c], xTo=xTo[c], dstloc=dstloc[c],
                 srcg=srcg[c], pa=pa[c], pb=pb[c], mtb=mtb[c])
        per_core.append(m)
    return per_core, t_fix, npt, pp


def _build(t_fix, npt):
    import concourse.bacc as bacc
    import concourse.mybir as mybir
    import concourse.tile as tile
    from concourse.bass import IndirectOffsetOnAxis

    F32 = mybir.dt.float32
    BF = mybir.dt.bfloat16
    AF = mybir.ActivationFunctionType
    OP = mybir.AluOpType
    T = t_fix
    nT = NB * T
    n_slots = nT * P

    nc = bacc.Bacc("TRN2", target_bir_lowering=False, debug=False, num_devices=NC)
    din = {}
    for name, shape, dt in [
        ("xe", [n_slots, DIM], F32), ("xeT", [DIM, n_slots], F32),
        ("mtb", [P, n_slots], F32),
        ("xTo", [DIM, BLKP], F32), ("dstloc", [P, nT], F32),
        ("srcg", [P, nT], mybir.dt.int32),
        ("pa", [P, npt], mybir.dt.int32), ("pb", [P, npt], mybir.dt.int32),
        ("W1T", [P, HEADS * HID], F32), ("wasT1", [P, HEADS], F32),
        ("wadT1", [P, HEADS], F32), ("b1T", [P, 8], F32),
        ("W2augP", [P, 8 * 130], F32), ("b2_bc", [P, EMB], F32),
        ("clsWT", [P, 4], F32), ("clsb_bc", [P, 4], F32),
        ("iota_f", [P, P], F32), ("ident", [P, P], F32),
    ]:
        din[name] = nc.dram_tensor(name, shape, dt, kind="ExternalInput")
    z_ext = nc.dram_tensor("z_out", [BLKP, EMB], F32, kind="ExternalOutput")
    lg_ext = nc.dram_tensor("lg_out", [P, NB * 4], F32, kind="ExternalOutput")
    pr_ext = nc.dram_tensor("pr_out", [P, npt], F32, kind="ExternalOutput")

    g_own = nc.dram_tensor("g_own", [BLKP, GW], F32)
    g_all = nc.dram_tensor("g_all", [NC * BLKP, GW], F32, addr_space="Shared")
    z_own = nc.dram_tensor("z_own", [BLKP, EMB], F32)
    z_all = nc.dram_tensor("z_all", [NC * BLKP, EMB], F32, addr_space="Shared")

    with tile.TileContext(nc) as tc:
        with tc.tile_pool(name="const", bufs=1) as cp:
            W1T = cp.tile_from(din["W1T"].ap())
            wasT1 = cp.tile_from(din["wasT1"].ap())
            wadT1 = cp.tile_from(din["wadT1"].ap())
            b1T = cp.tile_from(din["b1T"].ap())
            W2augP = cp.tile_from(din["W2augP"].ap())
            b2_bc = cp.tile_from(din["b2_bc"].ap())
            clsWT = cp.tile_from(din["clsWT"].ap())
            clsb_bc = cp.tile_from(din["clsb_bc"].ap())
            iota_f = cp.tile_from(din["iota_f"].ap())
            ident = cp.tile_from(din["ident"].ap())
            dstloc = cp.tile_from(din["dstloc"].ap())
            srcg = cp.tile_from(din["srcg"].ap())
            pa_t = cp.tile_from(din["pa"].ap())
            pb_t = cp.tile_from(din["pb"].ap())
            zl_s = cp.tile([1, P], F32)
            nc.vector.memset(zl_s[:], 0.0)
            zr_s = cp.tile([1, 512], F32)
            nc.vector.memset(zr_s[:], 0.0)
            ad2_sb = cp.tile([P, NB], F32)
            lg_acc = cp.tile([P, NB * 4], F32)
            pr_acc = cp.tile([P, npt], F32)

            # ---------------- L1 + g-table ----------------
            with tc.tile_pool(name="sb1", bufs=2) as sb, \
                 tc.tile_pool(name="sbm", bufs=3) as sbm, \
                 tc.tile_pool(name="ps_acc", bufs=2, space="PSUM") as ps_acc, \
                 tc.tile_pool(name="ps_tr", bufs=2, space="PSUM") as ps_tr, \
                 tc.tile_pool(name="ps_o", bufs=1, space="PSUM") as ps_o:
                for b in range(NB):
                    e0 = b * T * P
                    xe_b = sb.tile([P, T * DIM], F32, tag="xe")
                    nc.sync.dma_start(
                        xe_b[:].rearrange("p (t d) -> p t d", t=T),
                        din["xe"][e0:e0 + T * P, :].rearrange("(t p) d -> p t d", p=P))
                    xeT_b = sb.tile([P, T * P], F32, tag="xeT")
                    nc.sync.dma_start(xeT_b[:], din["xeT"][:, e0:e0 + T * P])
                    mt_b = sb.tile([P, T * P], F32, tag="mt")
                    nc.sync.dma_start(mt_b[:], din["mtb"][:, e0:e0 + T * P])
                    xTo_t = sbm.tile([P, P], F32, tag="xTo")
                    nc.sync.dma_start(xTo_t[:], din["xTo"][:, b * P:(b + 1) * P])

                    ad1_ps = ps_tr.tile([P, HEADS], F32, tag="asad", padded_shape=[P, 512])
                    nc.tensor.matmul(ad1_ps[:], lhsT=xTo_t[:], rhs=wadT1[:],
                                     start=True, stop=True)
                    ad1_s = sbm.tile([P, HEADS], F32, tag="ad1s")
                    nc.vector.tensor_copy(ad1_s[:], ad1_ps[:])

                    # batched as+ad for all T tiles: [e, 4*T]
                    asad_ps = ps_tr.tile([P, 4 * T], F32, tag="asad", padded_shape=[P, 512])
                    nc.tensor.matmul(asad_ps[:], lhsT=zl_s[:], rhs=zr_s[:, 0:4 * T],
                                     start=True, stop=False)
                    for j in range(T):
                        nc.tensor.matmul(asad_ps[:, 4 * j:4 * j + 4],
                                         lhsT=xeT_b[:, j * P:(j + 1) * P],
                                         rhs=wasT1[:], start=False, stop=False)
                        nc.tensor.matmul(asad_ps[:, 4 * j:4 * j + 4],
                                         lhsT=mt_b[:, j * P:(j + 1) * P],
                                         rhs=ad1_s[:], start=False,
                                         stop=(j == T - 1))
                    t02 = sbm.tile([P, 4 * T], F32, tag="t02")
                    nc.vector.tensor_scalar(out=t02[:], in0=asad_ps[:], scalar1=SLOPE,
                                            scalar2=None, op0=OP.mult)
                    e_s = sbm.tile([P, 4 * T], F32, tag="e")
                    nc.vector.tensor_tensor(out=e_s[:], in0=asad_ps[:], in1=t02[:],
                                            op=OP.max)
                    al_s = sbm.tile([P, 4 * T], F32, tag="al")
                    nc.scalar.activation(al_s[:], e_s[:], AF.Exp)

                    agg01 = ps_acc.tile([P, 258], F32, tag="agg01", padded_shape=[P, 512])
                    agg23 = ps_acc.tile([P, 258], F32, tag="agg23", padded_shape=[P, 512])
                    nc.tensor.matmul(agg01[:], lhsT=zl_s[:], rhs=zr_s[:, 0:258],
                                     start=True, stop=False)
                    nc.tensor.matmul(agg23[:], lhsT=zl_s[:], rhs=zr_s[:, 0:258],
                                     start=True, stop=False)
                    for j in range(T):
                        m_s = sbm.tile([P, P], F32, tag="m")
                        nc.vector.tensor_tensor(
                            out=m_s[:],
                            in0=dstloc[:, b * T + j:b * T + j + 1].to_broadcast([P, P]),
                            in1=iota_f[:], op=OP.is_equal)
                        xw01 = sbm.tile([P, 258], F32, tag="xw01")
                        xw23 = sbm.tile([P, 258], F32, tag="xw23")
                        for k in range(4):
                            dstt = (xw01, xw23)[k // 2]
                            nc.vector.tensor_scalar(
                                out=dstt[:, (k % 2) * P:(k % 2) * P + P],
                                in0=xe_b[:, j * DIM:(j + 1) * DIM],
                                scalar1=al_s[:, 4 * j + k:4 * j + k + 1],
                                scalar2=None, op0=OP.mult)
                        nc.vector.tensor_copy(xw01[:, 256:258],
                                              al_s[:, 4 * j:4 * j + 2])
                        nc.vector.tensor_copy(xw23[:, 256:258],
                                              al_s[:, 4 * j + 2:4 * j + 4])
                        last = (j == T - 1)
                        nc.tensor.matmul(agg01[:], lhsT=m_s[:], rhs=xw01[:],
                                         start=False, stop=last)
                        nc.tensor.matmul(agg23[:], lhsT=m_s[:], rhs=xw23[:],
                                         start=False, stop=last)

                    # block tail
                    denr = sbm.tile([P, HEADS], F32, tag="denr")
                    nc.vector.tensor_scalar(out=denr[:, 0:2], in0=agg01[:, 256:258],
                                            scalar1=1e-16, scalar2=None, op0=OP.add)
                    nc.vector.tensor_scalar(out=denr[:, 2:4], in0=agg23[:, 256:258],
                                            scalar1=1e-16, scalar2=None, op0=OP.add)
                    nc.vector.reciprocal(denr[:], denr[:])
                    aggn = sb.tile([P, 4 * P], F32, tag="aggn")
                    for k in range(4):
                        srct = (agg01, agg23)[k // 2]
                        nc.vector.tensor_scalar(
                            out=aggn[:, k * P:(k + 1) * P],
                            in0=srct[:, (k % 2) * P:(k % 2) * P + P],
                            scalar1=denr[:, k:k + 1], scalar2=None, op0=OP.mult)
                    aggT_ps = ps_o.tile([P, 4 * P], F32, tag="tail", padded_shape=[P, 1024])
                    for k in range(4):
                        nc.tensor.transpose(aggT_ps[:, k * P:(k + 1) * P],
                                            aggn[:, k * P:(k + 1) * P], ident[:])
                    aggT_s = sb.tile([P, 4 * P], F32, tag="aggTs")
                    nc.vector.tensor_copy(aggT_s[:], aggT_ps[:])

                    o1_ps = ps_o.tile([P, 8 * P], F32, tag="tail", padded_shape=[P, 1024])
                    for c in range(8):
                        nc.tensor.matmul(
                            o1_ps[:, c * P:(c + 1) * P],
                            lhsT=W1T[:, c * P:(c + 1) * P],
                            rhs=aggT_s[:, (c // 2) * P:(c // 2 + 1) * P],
                            start=True, stop=True)
                    h1T = sb.tile([P, 8 * P], F32, tag="h1T")
                    for c in range(8):
                        tt = sbm.tile([P, P], F32, tag="elu_t")
                        rr = sbm.tile([P, P], F32, tag="elu_r")
                        nc.scalar.activation(tt[:], o1_ps[:, c * P:(c + 1) * P],
                                             AF.Exp, bias=b1T[:, c:c + 1])
                        nc.scalar.activation(rr[:], o1_ps[:, c * P:(c + 1) * P],
                                             AF.Relu, bias=b1T[:, c:c + 1])
                        nc.vector.tensor_scalar(out=tt[:], in0=tt[:], scalar1=-1.0,
                                                scalar2=None, op0=OP.add)
                        nc.vector.tensor_tensor(out=h1T[:, c * P:(c + 1) * P],
                                                in0=tt[:], in1=rr[:], op=OP.min)
                    gaug_ps = ps_o.tile([P, 130], F32, tag="tail", padded_shape=[P, 1024])
                    for c in range(8):
                        nc.tensor.matmul(gaug_ps[:], lhsT=h1T[:, c * P:(c + 1) * P],
                                         rhs=W2augP[:, c * 130:(c + 1) * 130],
                                         start=(c == 0), stop=(c == 7))
                    g_sb = sb.tile([P, GW], F32, tag="gsb")
                    nc.vector.tensor_copy(g_sb[:, 0:128], gaug_ps[:, 0:128])
                    nc.vector.memset(g_sb[:, 128:129], 1.0)
                    nc.vector.tensor_copy(g_sb[:, 129:131], gaug_ps[:, 128:130])
                    nc.vector.tensor_copy(ad2_sb[:, b:b + 1], gaug_ps[:, 129:130])
                    nc.sync.dma_start(g_own[b * P:(b + 1) * P, :], g_sb[:])

            # ---------------- AllGather g ----------------
            nc.gpsimd.collective_compute(
                "AllGather", mybir.AluOpType.bypass,
                replica_groups=[list(range(NC))],
                ins=[g_own.ap().opt()], outs=[g_all.ap().opt()])

            # ---------------- L2 ----------------
            with tc.tile_pool(name="sb2", bufs=2) as sb, \
                 tc.tile_pool(name="sbg", bufs=2 * T + 2) as sbg, \
                 tc.tile_pool(name="sbm2", bufs=3) as sbm, \
                 tc.tile_pool(name="ps2_acc", bufs=2, space="PSUM") as ps_acc, \
                 tc.tile_pool(name="ps2_tr", bufs=2, space="PSUM") as ps_tr:
                for b in range(NB):
                    mt_b = sb.tile([P, T * P], F32, tag="mt2")
                    e0 = b * T * P
                    nc.sync.dma_start(mt_b[:], din["mtb"][:, e0:e0 + T * P])
                    g_ts = []
                    as2_all = sbm.tile([P, T], F32, tag="as2a")
                    e2_ps = ps_tr.tile([P, T], F32, tag="e2", padded_shape=[P, 512])
                    nc.tensor.matmul(e2_ps[:], lhsT=zl_s[:], rhs=zr_s[:, 0:T],
                                     start=True, stop=False)
                    for j in range(T):
                        g_t = sbg.tile([P, GW], F32, tag="gt")
                        nc.gpsimd.indirect_dma_start(
                            out=g_t[:], out_offset=None, in_=g_all[:, :],
                            in_offset=IndirectOffsetOnAxis(
                                ap=srcg[:, b * T + j:b * T + j + 1], axis=0))
                        g_ts.append(g_t)
                        nc.vector.tensor_copy(as2_all[:, j:j + 1], g_t[:, 129:130])
                        nc.tensor.matmul(e2_ps[:, j:j + 1],
                                         lhsT=mt_b[:, j * P:(j + 1) * P],
                                         rhs=ad2_sb[:, b:b + 1], start=False,
                                         stop=(j == T - 1))
                    s2_s = sbm.tile([P, T], F32, tag="s2s")
                    nc.vector.tensor_tensor(out=s2_s[:], in0=e2_ps[:], in1=as2_all[:],
                                            op=OP.add)
                    t2_s = sbm.tile([P, T], F32, tag="t2s")
                    nc.vector.tensor_scalar(out=t2_s[:], in0=s2_s[:], scalar1=SLOPE,
                                            scalar2=None, op0=OP.mult)
                    e2m = sbm.tile([P, T], F32, tag="e2m")
                    nc.vector.tensor_tensor(out=e2m[:], in0=s2_s[:], in1=t2_s[:],
                                            op=OP.max)
                    al2 = sbm.tile([P, T], F32, tag="al2")
                    nc.scalar.activation(al2[:], e2m[:], AF.Exp)

                    agg2_ps = ps_acc.tile([P, 129], F32, tag="agg2", padded_shape=[P, 512])
                    nc.tensor.matmul(agg2_ps[:], lhsT=zl_s[:], rhs=zr_s[:, 0:129],
                                     start=True, stop=False)
                    for j in range(T):
                        m_s = sbm.tile([P, P], F32, tag="m2")
                        nc.vector.tensor_tensor(
                            out=m_s[:],
                            in0=dstloc[:, b * T + j:b * T + j + 1].to_broadcast([P, P]),
                            in1=iota_f[:], op=OP.is_equal)
                        rhs2 = sbm.tile([P, 129], F32, tag="rhs2")
                        nc.vector.tensor_scalar(out=rhs2[:], in0=g_ts[j][:, 0:129],
                                                scalar1=al2[:, j:j + 1], scalar2=None,
                                                op0=OP.mult)
                        nc.tensor.matmul(agg2_ps[:], lhsT=m_s[:], rhs=rhs2[:],
                                         start=False, stop=(j == T - 1))
                    den2r = sbm.tile([P, 1], F32, tag="den2r")
                    nc.vector.tensor_scalar(out=den2r[:], in0=agg2_ps[:, 128:129],
                                            scalar1=1e-16, scalar2=None, op0=OP.add)
                    nc.vector.reciprocal(den2r[:], den2r[:])
                    y_s = sb.tile([P, EMB], F32, tag="y")
                    nc.vector.tensor_scalar(out=y_s[:], in0=agg2_ps[:, 0:128],
                                            scalar1=den2r[:, 0:1], scalar2=None,
                                            op0=OP.mult)
                    nc.vector.tensor_tensor(out=y_s[:], in0=y_s[:], in1=b2_bc[:],
                                            op=OP.add)
                    tt = sbm.tile([P, EMB], F32, tag="z_t")
                    rr = sbm.tile([P, EMB], F32, tag="z_r")
                    nc.scalar.activation(tt[:], y_s[:], AF.Exp)
                    nc.scalar.activation(rr[:], y_s[:], AF.Relu)
                    nc.vector.tensor_scalar(out=tt[:], in0=tt[:], scalar1=-1.0,
                                            scalar2=None, op0=OP.add)
                    z_sb = sb.tile([P, EMB], F32, tag="zsb")
                    nc.vector.tensor_tensor(out=z_sb[:], in0=tt[:], in1=rr[:], op=OP.min)
                    nc.sync.dma_start(z_own[b * P:(b + 1) * P, :], z_sb[:])
                    nc.sync.dma_start(z_ext[b * P:(b + 1) * P, :], z_sb[:])
                    zT_ps = ps_tr.tile([P, P], F32, tag="ztail", padded_shape=[P, 512])
                    nc.tensor.transpose(zT_ps[:], z_sb[:], ident[:])
                    zT_s = sbm.tile([P, P], F32, tag="zTs")
                    nc.vector.tensor_copy(zT_s[:], zT_ps[:])
                    lg_ps = ps_tr.tile([P, 4], F32, tag="ztail", padded_shape=[P, 512])
                    nc.tensor.matmul(lg_ps[:], lhsT=zT_s[:], rhs=clsWT[:],
                                     start=True, stop=True)
                    nc.vector.tensor_tensor(out=lg_acc[:, b * 4:(b + 1) * 4],
                                            in0=lg_ps[:], in1=clsb_bc[:], op=OP.add)

            # ---------------- AllGather z, preds ----------------
            nc.gpsimd.collective_compute(
                "AllGather", mybir.AluOpType.bypass,
                replica_groups=[list(range(NC))],
                ins=[z_own.ap().opt()], outs=[z_all.ap().opt()])

            with tc.tile_pool(name="sb3", bufs=4) as sb:
                sums = cp.tile([P, npt], F32)
                for q in range(npt):
                    za = sb.tile([P, EMB], F32, tag="za")
                    nc.gpsimd.indirect_dma_start(
                        out=za[:], out_offset=None, in_=z_all[:, :],
                        in_offset=IndirectOffsetOnAxis(ap=pa_t[:, q:q + 1], axis=0))
                    zb = sb.tile([P, EMB], F32, tag="zb")
                    nc.gpsimd.indirect_dma_start(
                        out=zb[:], out_offset=None, in_=z_all[:, :],
                        in_offset=IndirectOffsetOnAxis(ap=pb_t[:, q:q + 1], axis=0))
                    mm = sb.tile([P, EMB], F32, tag="zm")
                    nc.vector.tensor_tensor(out=mm[:], in0=za[:], in1=zb[:],
                                            op=OP.mult)
                    nc.vector.reduce_sum(out=sums[:, q:q + 1], in_=mm[:],
                                         axis=mybir.AxisListType.X)
                nc.scalar.activation(pr_acc[:], sums[:], AF.Sigmoid)
            nc.sync.dma_start(lg_ext[:, :], lg_acc[:])
            nc.sync.dma_start(pr_ext[:, :], pr_acc[:])
    nc.compile()
    return nc


_LAST_EXEC_NS = None


def _trace_kwargs():
    import os
    if not os.environ.get("GAT_TRACE"):
        return {}
    import sys, types
    try:
        import antenv.axon_hooks  # noqa: F401
    except ImportError:
        import antenv
        mod = types.ModuleType("antenv.axon_hooks")
        mod._hook = None
        mod.set_axon_ntff_profile_hook = lambda h: setattr(mod, "_hook", h)
        mod.get_axon_ntff_profile_hook = lambda: mod._hook
        sys.modules["antenv.axon_hooks"] = mod
        antenv.axon_hooks = mod
        try:
            from trn_agent_boot.trn_boot import _ntff_profile_via_ctypes
            mod.set_axon_ntff_profile_hook(
                _ntff_profile_via_ctypes("/opt/axon/libaxon_pjrt.so"))
        except Exception:
            return {}
    return {"trace": True}


def kernel(**inputs):
    global _LAST_EXEC_NS
    from concourse.bass_utils import run_bass_kernel_spmd
    per_core, t_fix, npt, pp = _prep(**inputs)
    nc = _build(t_fix, npt)
    res = run_bass_kernel_spmd(nc, per_core, core_ids=list(range(NC)),
                               **_trace_kwargs())
    _LAST_EXEC_NS = res.exec_time_ns
    z = np.concatenate([res.results[c]["z_out"][:BLK] for c in range(NC)], axis=0)
    logits = np.concatenate(
        [res.results[c]["lg_out"].reshape(P, NB, 4).transpose(1, 0, 2)
         .reshape(BLKP, 4)[:BLK] for c in range(NC)], axis=0)
    preds = np.concatenate(
        [res.results[c]["pr_out"].T.reshape(npt * P)[:pp] for c in range(NC)])
    return z.astype(np.float32), logits.astype(np.float32), preds.astype(np.float32)


# revision 15
# speedup vs baseline: 1.0365x; 1.0365x over previous
"""2-layer GAT (GATConv x2 + link predictor) on 8 Trainium2 NeuronCores.

Sharding: nodes partitioned into 8 contiguous blocks (graph parallel).
Each core aggregates incoming edges of its own destination block.
L1 source features are host-pre-gathered into edge-slot order (x is an
input); L2 aggregates the device-computed g-table (AllGather'd across
cores) with device-side indirect-DMA gathers, and link predictions
gather rows of the AllGather'd z-table.

Segment softmax / segment sum are done with per-tile mask matmuls on the
tensor engine: edges are sorted by destination and padded per 128-node
destination tile; M[e,d] = (dstloc[e]==d) maps 128 edges onto the tile's
nodes, so M.T @ (alpha * feat) accumulates per-node sums in PSUM.
"""
import numpy as np
import ml_dtypes

BF16 = ml_dtypes.bfloat16

# model dims (from the reference problem; fixed by the harness)
N_NODES = 50000
DIM = 128
HEADS = 4
HID = 256
EMB = 128
SLOPE = 0.2

NC = 8
P = 128
NB = 50                        # dst tiles (node groups) per core
GRP = N_NODES // (NC * NB)     # 125 real nodes per group (3 pad slots)
BLKP = NB * P                  # 6400 padded rows per core
GW = 131                       # g-table row: [g(128) | 1.0 | as2 | ad2]
PAD_DST = 999.0


def _prep(x, e, p, n, W1, a_src1, a_dst1, b1, W2, a_src2, a_dst2, b2, cls_W, cls_b):
    """Host-side graph/index/layout prep. No model FLOPs on node data."""
    x = np.asarray(x, np.float32)
    e = np.asarray(e, np.int64)
    p = np.asarray(p, np.int64)
    n = np.asarray(n, np.int64)

    loop = np.arange(N_NODES, dtype=np.int64)
    src = np.concatenate([e[0], loop])
    dst = np.concatenate([e[1], loop])

    # degree-balanced node grouping: snake-assign nodes (sorted by in-degree)
    # to NC*NB groups of GRP nodes so per-group edge counts are uniform.
    deg = np.bincount(dst, minlength=N_NODES)
    by_deg = np.argsort(-deg, kind="stable")
    ngrp = NC * NB
    grp = np.zeros(N_NODES, np.int64)
    pos = np.zeros(N_NODES, np.int64)
    for pss in range(GRP):
        seg = by_deg[pss * ngrp:(pss + 1) * ngrp]
        gids = np.arange(ngrp) if pss % 2 == 0 else np.arange(ngrp)[::-1]
        grp[seg] = gids
        pos[seg] = pss
    rowv = grp * P + pos                      # node -> padded table row
    row_of = lambda g: rowv[g]

    # per (core, dst-tile) edge lists
    key = grp[dst]
    core_of = key // NB
    order = np.lexsort((dst, key))
    src_s, dst_s = src[order], dst[order]
    counts = np.bincount(key, minlength=NC * NB)
    t_fix = int(np.max((counts + P - 1) // P))
    nT = NB * t_fix
    n_slots = nT * P

    starts = np.zeros(NC * NB + 1, dtype=np.int64)
    np.cumsum(counts, out=starts[1:])

    xe = np.zeros((NC, n_slots, DIM), np.float32)
    dstloc_sl = np.full((NC, n_slots), PAD_DST, np.float32)
    srcg_sl = np.zeros((NC, n_slots), np.int32)
    for c in range(NC):
        for b in range(NB):
            k = c * NB + b
            cnt = counts[k]
            if cnt == 0:
                continue
            s0 = starts[k]
            base = b * t_fix * P
            sl = slice(base, base + cnt)
            es = src_s[s0:s0 + cnt]
            xe[c, sl] = x[es]
            dstloc_sl[c, sl] = pos[dst_s[s0:s0 + cnt]]
            srcg_sl[c, sl] = rowv[es]
    # slot s lives at [partition s%128, tile s//128]
    dstloc = dstloc_sl.reshape(NC, nT, P).transpose(0, 2, 1).copy()   # [NC,P,nT]
    srcg = srcg_sl.reshape(NC, nT, P).transpose(0, 2, 1).copy()
    xeT = np.ascontiguousarray(xe.transpose(0, 2, 1))                 # [NC,DIM,n_slots]
    # transposed masks MT[d, e] per tile, laid out [P, nT*P]
    dl = dstloc_sl.reshape(NC, nT, P)                                 # [c, t, e]
    mtb = (dl[:, :, None, :] == np.arange(P, dtype=np.float32)[None, None, :, None])
    mtb = np.ascontiguousarray(
        mtb.astype(np.float32).transpose(0, 2, 1, 3).reshape(NC, P, nT * P))

    # own-block x transposed (relabeled rows), padded
    xall = np.zeros((NC * BLKP, DIM), np.float32)
    xall[rowv] = x
    xTo = np.zeros((NC, DIM, BLKP), np.float32)
    for c in range(NC):
        xTo[c] = xall[c * BLKP:(c + 1) * BLKP].T

    # pred edges sharded by position
    n_pred = p.shape[1] + n.shape[1]
    pp = n_pred // NC
    npt = (pp + P - 1) // P
    pa = np.zeros((NC, npt * P), np.int32)
    pb = np.zeros((NC, npt * P), np.int32)
    allp = np.concatenate([p, n], axis=1)
    for c in range(NC):
        seg = allp[:, c * pp:(c + 1) * pp]
        pa[c, :pp] = row_of(seg[0])
        pb[c, :pp] = row_of(seg[1])
    pa = pa.reshape(NC, npt, P).transpose(0, 2, 1).copy()
    pb = pb.reshape(NC, npt, P).transpose(0, 2, 1).copy()

    # weight prep (weights only)
    W1 = np.asarray(W1, np.float32)
    W2 = np.asarray(W2, np.float32)
    W1h = W1.reshape(HEADS, HID, DIM)
    was1 = np.einsum('kh,khd->kd', np.asarray(a_src1, np.float32), W1h)
    wad1 = np.einsum('kh,khd->kd', np.asarray(a_dst1, np.float32), W1h)
    W1T = np.ascontiguousarray(W1.T)                                   # [128, 1024]
    b1 = np.asarray(b1, np.float32)
    b1T = b1.reshape(8, P).T.copy()                                    # [128, 8] col c
    W2T = np.ascontiguousarray(W2.T)                                   # [1024, 128]
    was2 = W2T @ np.asarray(a_src2, np.float32)[0]                     # [1024]
    wad2 = W2T @ np.asarray(a_dst2, np.float32)[0]
    W2aug = np.zeros((8, P, 130), np.float32)
    for c in range(8):
        W2aug[c, :, :128] = W2T[c * P:(c + 1) * P]
        W2aug[c, :, 128] = was2[c * P:(c + 1) * P]
        W2aug[c, :, 129] = wad2[c * P:(c + 1) * P]
    W2augP = np.ascontiguousarray(W2aug.transpose(1, 0, 2)).reshape(P, 8 * 130)
    b2_bc = np.tile(np.asarray(b2, np.float32)[None, :], (P, 1))       # [128, 128]
    clsWT = np.ascontiguousarray(np.asarray(cls_W, np.float32).T)      # [128, 4]
    clsb_bc = np.tile(np.asarray(cls_b, np.float32)[None, :], (P, 1))  # [128, 4]
    iota_f = np.tile(np.arange(P, dtype=np.float32)[None, :], (P, 1))
    ident = np.eye(P, dtype=np.float32)

    shared = dict(W1T=W1T, wasT1=np.ascontiguousarray(was1.T),
                  wadT1=np.ascontiguousarray(wad1.T), b1T=b1T,
                  W2augP=W2augP, b2_bc=b2_bc, clsWT=clsWT, clsb_bc=clsb_bc,
                  iota_f=iota_f, ident=ident)
    per_core = []
    for c in range(NC):
        m = dict(shared)
        m.update(xe=xe[c], xeT=xeT[c], xTo=xTo[c], dstloc=dstloc[c],
                 srcg=srcg[c], pa=pa[c], pb=pb[c], mtb=mtb[c])
        per_core.append(m)
    return per_core, t_fix, npt, pp, rowv


def _build(t_fix, npt):
    import concourse.bacc as bacc
    import concourse.mybir as mybir
    import concourse.tile as tile
    from concourse.bass import IndirectOffsetOnAxis

    F32 = mybir.dt.float32
    BF = mybir.dt.bfloat16
    AF = mybir.ActivationFunctionType
    OP = mybir.AluOpType
    T = t_fix
    nT = NB * T
    n_slots = nT * P

    nc = bacc.Bacc("TRN2", target_bir_lowering=False, debug=False, num_devices=NC)
    din = {}
    for name, shape, dt in [
        ("xe", [n_slots, DIM], F32), ("xeT", [DIM, n_slots], F32),
        ("mtb", [P, n_slots], F32),
        ("xTo", [DIM, BLKP], F32), ("dstloc", [P, nT], F32),
        ("srcg", [P, nT], mybir.dt.int32),
        ("pa", [P, npt], mybir.dt.int32), ("pb", [P, npt], mybir.dt.int32),
        ("W1T", [P, HEADS * HID], F32), ("wasT1", [P, HEADS], F32),
        ("wadT1", [P, HEADS], F32), ("b1T", [P, 8], F32),
        ("W2augP", [P, 8 * 130], F32), ("b2_bc", [P, EMB], F32),
        ("clsWT", [P, 4], F32), ("clsb_bc", [P, 4], F32),
        ("iota_f", [P, P], F32), ("ident", [P, P], F32),
    ]:
        din[name] = nc.dram_tensor(name, shape, dt, kind="ExternalInput")
    z_ext = nc.dram_tensor("z_out", [BLKP, EMB], F32, kind="ExternalOutput")
    lg_ext = nc.dram_tensor("lg_out", [P, NB * 4], F32, kind="ExternalOutput")
    pr_ext = nc.dram_tensor("pr_out", [P, npt], F32, kind="ExternalOutput")

    g_own = nc.dram_tensor("g_own", [BLKP, GW], F32)
    g_all = nc.dram_tensor("g_all", [NC * BLKP, GW], F32, addr_space="Shared")
    z_own = nc.dram_tensor("z_own", [BLKP, EMB], F32)
    z_all = nc.dram_tensor("z_all", [NC * BLKP, EMB], F32, addr_space="Shared")

    with tile.TileContext(nc) as tc:
        with tc.tile_pool(name="const", bufs=1) as cp:
            W1T = cp.tile_from(din["W1T"].ap())
            wasT1 = cp.tile_from(din["wasT1"].ap())
            wadT1 = cp.tile_from(din["wadT1"].ap())
            b1T = cp.tile_from(din["b1T"].ap())
            W2augP = cp.tile_from(din["W2augP"].ap())
            b2_bc = cp.tile_from(din["b2_bc"].ap())
            clsWT = cp.tile_from(din["clsWT"].ap())
            clsb_bc = cp.tile_from(din["clsb_bc"].ap())
            iota_f = cp.tile_from(din["iota_f"].ap())
            ident = cp.tile_from(din["ident"].ap())
            dstloc = cp.tile_from(din["dstloc"].ap())
            srcg = cp.tile_from(din["srcg"].ap())
            pa_t = cp.tile_from(din["pa"].ap())
            pb_t = cp.tile_from(din["pb"].ap())
            zl_s = cp.tile([1, P], F32)
            nc.vector.memset(zl_s[:], 0.0)
            zr_s = cp.tile([1, 512], F32)
            nc.vector.memset(zr_s[:], 0.0)
            ad2_sb = cp.tile([P, NB], F32)
            lg_acc = cp.tile([P, NB * 4], F32)
            pr_acc = cp.tile([P, npt], F32)

            # ---------------- L1 + g-table ----------------
            with tc.tile_pool(name="sb1", bufs=2) as sb, \
                 tc.tile_pool(name="sbm", bufs=3) as sbm, \
                 tc.tile_pool(name="ps_acc", bufs=2, space="PSUM") as ps_acc, \
                 tc.tile_pool(name="ps_tr", bufs=2, space="PSUM") as ps_tr, \
                 tc.tile_pool(name="ps_o", bufs=1, space="PSUM") as ps_o:
                for b in range(NB):
                    e0 = b * T * P
                    xe_b = sb.tile([P, T * DIM], F32, tag="xe")
                    nc.sync.dma_start(
                        xe_b[:].rearrange("p (t d) -> p t d", t=T),
                        din["xe"][e0:e0 + T * P, :].rearrange("(t p) d -> p t d", p=P))
                    xeT_b = sb.tile([P, T * P], F32, tag="xeT")
                    nc.sync.dma_start(xeT_b[:], din["xeT"][:, e0:e0 + T * P])
                    mt_b = sb.tile([P, T * P], F32, tag="mt")
                    nc.sync.dma_start(mt_b[:], din["mtb"][:, e0:e0 + T * P])
                    xTo_t = sbm.tile([P, P], F32, tag="xTo")
                    nc.sync.dma_start(xTo_t[:], din["xTo"][:, b * P:(b + 1) * P])

                    ad1_ps = ps_tr.tile([P, HEADS], F32, tag="asad", padded_shape=[P, 512])
                    nc.tensor.matmul(ad1_ps[:], lhsT=xTo_t[:], rhs=wadT1[:],
                                     start=True, stop=True)
                    ad1_s = sbm.tile([P, HEADS], F32, tag="ad1s")
                    nc.vector.tensor_copy(ad1_s[:], ad1_ps[:])

                    # batched as+ad for all T tiles: [e, 4*T]
                    asad_ps = ps_tr.tile([P, 4 * T], F32, tag="asad", padded_shape=[P, 512])
                    nc.tensor.matmul(asad_ps[:], lhsT=zl_s[:], rhs=zr_s[:, 0:4 * T],
                                     start=True, stop=False)
                    for j in range(T):
                        nc.tensor.matmul(asad_ps[:, 4 * j:4 * j + 4],
                                         lhsT=xeT_b[:, j * P:(j + 1) * P],
                                         rhs=wasT1[:], start=False, stop=False)
                        nc.tensor.matmul(asad_ps[:, 4 * j:4 * j + 4],
                                         lhsT=mt_b[:, j * P:(j + 1) * P],
                                         rhs=ad1_s[:], start=False,
                                         stop=(j == T - 1))
                    t02 = sbm.tile([P, 4 * T], F32, tag="t02")
                    nc.vector.tensor_scalar(out=t02[:], in0=asad_ps[:], scalar1=SLOPE,
                                            scalar2=None, op0=OP.mult)
                    e_s = sbm.tile([P, 4 * T], F32, tag="e")
                    nc.vector.tensor_tensor(out=e_s[:], in0=asad_ps[:], in1=t02[:],
                                            op=OP.max)
                    al_s = sbm.tile([P, 4 * T], F32, tag="al")
                    nc.scalar.activation(al_s[:], e_s[:], AF.Exp)

                    agg01 = ps_acc.tile([P, 258], F32, tag="agg01", padded_shape=[P, 512])
                    agg23 = ps_acc.tile([P, 258], F32, tag="agg23", padded_shape=[P, 512])
                    nc.tensor.matmul(agg01[:], lhsT=zl_s[:], rhs=zr_s[:, 0:258],
                                     start=True, stop=False)
                    nc.tensor.matmul(agg23[:], lhsT=zl_s[:], rhs=zr_s[:, 0:258],
                                     start=True, stop=False)
                    for j in range(T):
                        m_s = sbm.tile([P, P], F32, tag="m")
                        nc.vector.tensor_tensor(
                            out=m_s[:],
                            in0=dstloc[:, b * T + j:b * T + j + 1].to_broadcast([P, P]),
                            in1=iota_f[:], op=OP.is_equal)
                        xw01 = sbm.tile([P, 258], F32, tag="xw01")
                        xw23 = sbm.tile([P, 258], F32, tag="xw23")
                        for k in range(4):
                            dstt = (xw01, xw23)[k // 2]
                            nc.vector.tensor_scalar(
                                out=dstt[:, (k % 2) * P:(k % 2) * P + P],
                                in0=xe_b[:, j * DIM:(j + 1) * DIM],
                                scalar1=al_s[:, 4 * j + k:4 * j + k + 1],
                                scalar2=None, op0=OP.mult)
                        nc.vector.tensor_copy(xw01[:, 256:258],
                                              al_s[:, 4 * j:4 * j + 2])
                        nc.vector.tensor_copy(xw23[:, 256:258],
                                              al_s[:, 4 * j + 2:4 * j + 4])
                        last = (j == T - 1)
                        nc.tensor.matmul(agg01[:], lhsT=m_s[:], rhs=xw01[:],
                                         start=False, stop=last)
                        nc.tensor.matmul(agg23[:], lhsT=m_s[:], rhs=xw23[:],
                                         start=False, stop=last)

                    # block tail
                    denr = sbm.tile([P, HEADS], F32, tag="denr")
                    nc.vector.tensor_scalar(out=denr[:, 0:2], in0=agg01[:, 256:258],
                                            scalar1=1e-16, scalar2=None, op0=OP.add)
                    nc.vector.tensor_scalar(out=denr[:, 2:4], in0=agg23[:, 256:258],
                                            scalar1=1e-16, scalar2=None, op0=OP.add)
                    nc.vector.reciprocal(denr[:], denr[:])
                    aggn = sb.tile([P, 4 * P], F32, tag="aggn")
                    for k in range(4):
                        srct = (agg01, agg23)[k // 2]
                        nc.vector.tensor_scalar(
                            out=aggn[:, k * P:(k + 1) * P],
                            in0=srct[:, (k % 2) * P:(k % 2) * P + P],
                            scalar1=denr[:, k:k + 1], scalar2=None, op0=OP.mult)
                    aggT_ps = ps_o.tile([P, 4 * P], F32, tag="tail", padded_shape=[P, 1024])
                    for k in range(4):
                        nc.tensor.transpose(aggT_ps[:, k * P:(k + 1) * P],
                                            aggn[:, k * P:(k + 1) * P], ident[:])
                    aggT_s = sb.tile([P, 4 * P], F32, tag="aggTs")
                    nc.vector.tensor_copy(aggT_s[:], aggT_ps[:])

                    o1_ps = ps_o.tile([P, 8 * P], F32, tag="tail", padded_shape=[P, 1024])
                    for c in range(8):
                        nc.tensor.matmul(
                            o1_ps[:, c * P:(c + 1) * P],
                            lhsT=W1T[:, c * P:(c + 1) * P],
                            rhs=aggT_s[:, (c // 2) * P:(c // 2 + 1) * P],
                            start=True, stop=True)
                    h1T = sb.tile([P, 8 * P], F32, tag="h1T")
                    for c in range(8):
                        tt = sbm.tile([P, P], F32, tag="elu_t")
                        rr = sbm.tile([P, P], F32, tag="elu_r")
                        nc.scalar.activation(tt[:], o1_ps[:, c * P:(c + 1) * P],
                                             AF.Exp, bias=b1T[:, c:c + 1])
                        nc.scalar.activation(rr[:], o1_ps[:, c * P:(c + 1) * P],
                                             AF.Relu, bias=b1T[:, c:c + 1])
                        nc.vector.tensor_scalar(out=tt[:], in0=tt[:], scalar1=-1.0,
                                                scalar2=None, op0=OP.add)
                        nc.vector.tensor_tensor(out=h1T[:, c * P:(c + 1) * P],
                                                in0=tt[:], in1=rr[:], op=OP.min)
                    gaug_ps = ps_o.tile([P, 130], F32, tag="tail", padded_shape=[P, 1024])
                    for c in range(8):
                        nc.tensor.matmul(gaug_ps[:], lhsT=h1T[:, c * P:(c + 1) * P],
                                         rhs=W2augP[:, c * 130:(c + 1) * 130],
                                         start=(c == 0), stop=(c == 7))
                    g_sb = sb.tile([P, GW], F32, tag="gsb")
                    nc.vector.tensor_copy(g_sb[:, 0:128], gaug_ps[:, 0:128])
                    nc.vector.memset(g_sb[:, 128:129], 1.0)
                    nc.vector.tensor_copy(g_sb[:, 129:131], gaug_ps[:, 128:130])
                    nc.vector.tensor_copy(ad2_sb[:, b:b + 1], gaug_ps[:, 129:130])
                    nc.sync.dma_start(g_own[b * P:(b + 1) * P, :], g_sb[:])

            # ---------------- AllGather g ----------------
            nc.gpsimd.collective_compute(
                "AllGather", mybir.AluOpType.bypass,
                replica_groups=[list(range(NC))],
                ins=[g_own.ap().opt()], outs=[g_all.ap().opt()])

            # ---------------- L2 ----------------
            with tc.tile_pool(name="sb2", bufs=2) as sb, \
                 tc.tile_pool(name="sbg", bufs=2 * T + 2) as sbg, \
                 tc.tile_pool(name="sbm2", bufs=3) as sbm, \
                 tc.tile_pool(name="ps2_acc", bufs=2, space="PSUM") as ps_acc, \
                 tc.tile_pool(name="ps2_tr", bufs=2, space="PSUM") as ps_tr:
                for b in range(NB):
                    mt_b = sb.tile([P, T * P], F32, tag="mt2")
                    e0 = b * T * P
                    nc.sync.dma_start(mt_b[:], din["mtb"][:, e0:e0 + T * P])
                    g_ts = []
                    as2_all = sbm.tile([P, T], F32, tag="as2a")
                    e2_ps = ps_tr.tile([P, T], F32, tag="e2", padded_shape=[P, 512])
                    nc.tensor.matmul(e2_ps[:], lhsT=zl_s[:], rhs=zr_s[:, 0:T],
                                     start=True, stop=False)
                    for j in range(T):
                        g_t = sbg.tile([P, GW], F32, tag="gt")
                        nc.gpsimd.indirect_dma_start(
                            out=g_t[:], out_offset=None, in_=g_all[:, :],
                            in_offset=IndirectOffsetOnAxis(
                                ap=srcg[:, b * T + j:b * T + j + 1], axis=0))
                        g_ts.append(g_t)
                        nc.vector.tensor_copy(as2_all[:, j:j + 1], g_t[:, 129:130])
                        nc.tensor.matmul(e2_ps[:, j:j + 1],
                                         lhsT=mt_b[:, j * P:(j + 1) * P],
                                         rhs=ad2_sb[:, b:b + 1], start=False,
                                         stop=(j == T - 1))
                    s2_s = sbm.tile([P, T], F32, tag="s2s")
                    nc.vector.tensor_tensor(out=s2_s[:], in0=e2_ps[:], in1=as2_all[:],
                                            op=OP.add)
                    t2_s = sbm.tile([P, T], F32, tag="t2s")
                    nc.vector.tensor_scalar(out=t2_s[:], in0=s2_s[:], scalar1=SLOPE,
                                            scalar2=None, op0=OP.mult)
                    e2m = sbm.tile([P, T], F32, tag="e2m")
                    nc.vector.tensor_tensor(out=e2m[:], in0=s2_s[:], in1=t2_s[:],
                                            op=OP.max)
                    al2 = sbm.tile([P, T], F32, tag="al2")
                    nc.scalar.activation(al2[:], e2m[:], AF.Exp)

                    agg2_ps = ps_acc.tile([P, 129], F32, tag="agg2", padded_shape=[P, 512])
                    nc.tensor.matmul(agg2_ps[:], lhsT=zl_s[:], rhs=zr_s[:, 0:129],
                                     start=True, stop=False)
                    for j in range(T):
                        m_s = sbm.tile([P, P], F32, tag="m2")
                        nc.vector.tensor_tensor(
                            out=m_s[:],
                            in0=dstloc[:, b * T + j:b * T + j + 1].to_broadcast([P, P]),
                            in1=iota_f[:], op=OP.is_equal)
                        rhs2 = sbm.tile([P, 129], F32, tag="rhs2")
                        nc.vector.tensor_scalar(out=rhs2[:], in0=g_ts[j][:, 0:129],
                                                scalar1=al2[:, j:j + 1], scalar2=None,
                                                op0=OP.mult)
                        nc.tensor.matmul(agg2_ps[:], lhsT=m_s[:], rhs=rhs2[:],
                                         start=False, stop=(j == T - 1))
                    den2r = sbm.tile([P, 1], F32, tag="den2r")
                    nc.vector.tensor_scalar(out=den2r[:], in0=agg2_ps[:, 128:129],
                                            scalar1=1e-16, scalar2=None, op0=OP.add)
                    nc.vector.reciprocal(den2r[:], den2r[:])
                    y_s = sb.tile([P, EMB], F32, tag="y")
                    nc.vector.tensor_scalar(out=y_s[:], in0=agg2_ps[:, 0:128],
                                            scalar1=den2r[:, 0:1], scalar2=None,
                                            op0=OP.mult)
                    nc.vector.tensor_tensor(out=y_s[:], in0=y_s[:], in1=b2_bc[:],
                                            op=OP.add)
                    tt = sbm.tile([P, EMB], F32, tag="z_t")
                    rr = sbm.tile([P, EMB], F32, tag="z_r")
                    nc.scalar.activation(tt[:], y_s[:], AF.Exp)
                    nc.scalar.activation(rr[:], y_s[:], AF.Relu)
                    nc.vector.tensor_scalar(out=tt[:], in0=tt[:], scalar1=-1.0,
                                            scalar2=None, op0=OP.add)
                    z_sb = sb.tile([P, EMB], F32, tag="zsb")
                    nc.vector.tensor_tensor(out=z_sb[:], in0=tt[:], in1=rr[:], op=OP.min)
                    nc.sync.dma_start(z_own[b * P:(b + 1) * P, :], z_sb[:])
                    nc.sync.dma_start(z_ext[b * P:(b + 1) * P, :], z_sb[:])
                    zT_ps = ps_tr.tile([P, P], F32, tag="ztail", padded_shape=[P, 512])
                    nc.tensor.transpose(zT_ps[:], z_sb[:], ident[:])
                    zT_s = sbm.tile([P, P], F32, tag="zTs")
                    nc.vector.tensor_copy(zT_s[:], zT_ps[:])
                    lg_ps = ps_tr.tile([P, 4], F32, tag="ztail", padded_shape=[P, 512])
                    nc.tensor.matmul(lg_ps[:], lhsT=zT_s[:], rhs=clsWT[:],
                                     start=True, stop=True)
                    nc.vector.tensor_tensor(out=lg_acc[:, b * 4:(b + 1) * 4],
                                            in0=lg_ps[:], in1=clsb_bc[:], op=OP.add)

            # ---------------- AllGather z, preds ----------------
            nc.gpsimd.collective_compute(
                "AllGather", mybir.AluOpType.bypass,
                replica_groups=[list(range(NC))],
                ins=[z_own.ap().opt()], outs=[z_all.ap().opt()])

            with tc.tile_pool(name="sb3", bufs=4) as sb:
                sums = cp.tile([P, npt], F32)
                for q in range(npt):
                    za = sb.tile([P, EMB], F32, tag="za")
                    nc.gpsimd.indirect_dma_start(
                        out=za[:], out_offset=None, in_=z_all[:, :],
                        in_offset=IndirectOffsetOnAxis(ap=pa_t[:, q:q + 1], axis=0))
                    zb = sb.tile([P, EMB], F32, tag="zb")
                    nc.gpsimd.indirect_dma_start(
                        out=zb[:], out_offset=None, in_=z_all[:, :],
                        in_offset=IndirectOffsetOnAxis(ap=pb_t[:, q:q + 1], axis=0))
                    mm = sb.tile([P, EMB], F32, tag="zm")
                    nc.vector.tensor_tensor(out=mm[:], in0=za[:], in1=zb[:],
                                            op=OP.mult)
                    nc.vector.reduce_sum(out=sums[:, q:q + 1], in_=mm[:],
                                         axis=mybir.AxisListType.X)
                nc.scalar.activation(pr_acc[:], sums[:], AF.Sigmoid)
            nc.sync.dma_start(lg_ext[:, :], lg_acc[:])
            nc.sync.dma_start(pr_ext[:, :], pr_acc[:])
    nc.compile()
    return nc


_LAST_EXEC_NS = None


def _trace_kwargs():
    import os
    if not os.environ.get("GAT_TRACE"):
        return {}
    import sys, types
    try:
        import antenv.axon_hooks  # noqa: F401
    except ImportError:
        import antenv
        mod = types.ModuleType("antenv.axon_hooks")
        mod._hook = None
        mod.set_axon_ntff_profile_hook = lambda h: setattr(mod, "_hook", h)
        mod.get_axon_ntff_profile_hook = lambda: mod._hook
        sys.modules["antenv.axon_hooks"] = mod
        antenv.axon_hooks = mod
        try:
            from trn_agent_boot.trn_boot import _ntff_profile_via_ctypes
            mod.set_axon_ntff_profile_hook(
                _ntff_profile_via_ctypes("/opt/axon/libaxon_pjrt.so"))
        except Exception:
            return {}
    return {"trace": True}


def kernel(**inputs):
    global _LAST_EXEC_NS
    from concourse.bass_utils import run_bass_kernel_spmd
    per_core, t_fix, npt, pp, rowv = _prep(**inputs)
    nc = _build(t_fix, npt)
    res = run_bass_kernel_spmd(nc, per_core, core_ids=list(range(NC)),
                               **_trace_kwargs())
    _LAST_EXEC_NS = res.exec_time_ns
    z_cat = np.concatenate([res.results[c]["z_out"] for c in range(NC)], axis=0)
    lg_cat = np.concatenate(
        [res.results[c]["lg_out"].reshape(P, NB, 4).transpose(1, 0, 2)
         .reshape(BLKP, 4) for c in range(NC)], axis=0)
    z = z_cat[rowv]
    logits = lg_cat[rowv]
    preds = np.concatenate(
        [res.results[c]["pr_out"].T.reshape(npt * P)[:pp] for c in range(NC)])
    return z.astype(np.float32), logits.astype(np.float32), preds.astype(np.float32)


# revision 16
# speedup vs baseline: 1.0371x; 1.0006x over previous
"""2-layer GAT (GATConv x2 + link predictor) on 8 Trainium2 NeuronCores.

Sharding: nodes partitioned into 8 contiguous blocks (graph parallel).
Each core aggregates incoming edges of its own destination block.
L1 source features are host-pre-gathered into edge-slot order (x is an
input); L2 aggregates the device-computed g-table (AllGather'd across
cores) with device-side indirect-DMA gathers, and link predictions
gather rows of the AllGather'd z-table.

Segment softmax / segment sum are done with per-tile mask matmuls on the
tensor engine: edges are sorted by destination and padded per 128-node
destination tile; M[e,d] = (dstloc[e]==d) maps 128 edges onto the tile's
nodes, so M.T @ (alpha * feat) accumulates per-node sums in PSUM.
"""
import numpy as np

# model dims (from the reference problem; fixed by the harness)
N_NODES = 50000
DIM = 128
HEADS = 4
HID = 256
EMB = 128
SLOPE = 0.2

NC = 8
P = 128
NB = 50                        # dst tiles (node groups) per core
GRP = N_NODES // (NC * NB)     # 125 real nodes per group (3 pad slots)
BLKP = NB * P                  # 6400 padded rows per core
GW = 131                       # g-table row: [g(128) | 1.0 | as2 | ad2]
PAD_DST = 999.0


def _prep(x, e, p, n, W1, a_src1, a_dst1, b1, W2, a_src2, a_dst2, b2, cls_W, cls_b):
    """Host-side graph/index/layout prep. No model FLOPs on node data."""
    x = np.asarray(x, np.float32)
    e = np.asarray(e, np.int64)
    p = np.asarray(p, np.int64)
    n = np.asarray(n, np.int64)

    loop = np.arange(N_NODES, dtype=np.int64)
    src = np.concatenate([e[0], loop])
    dst = np.concatenate([e[1], loop])

    # degree-balanced node grouping: snake-assign nodes (sorted by in-degree)
    # to NC*NB groups of GRP nodes so per-group edge counts are uniform.
    deg = np.bincount(dst, minlength=N_NODES)
    by_deg = np.argsort(-deg, kind="stable")
    ngrp = NC * NB
    grp = np.zeros(N_NODES, np.int64)
    pos = np.zeros(N_NODES, np.int64)
    for pss in range(GRP):
        seg = by_deg[pss * ngrp:(pss + 1) * ngrp]
        gids = np.arange(ngrp) if pss % 2 == 0 else np.arange(ngrp)[::-1]
        grp[seg] = gids
        pos[seg] = pss
    rowv = grp * P + pos                      # node -> padded table row
    row_of = lambda g: rowv[g]

    # per (core, dst-tile) edge lists
    key = grp[dst]
    core_of = key // NB
    order = np.lexsort((dst, key))
    src_s, dst_s = src[order], dst[order]
    counts = np.bincount(key, minlength=NC * NB)
    t_fix = int(np.max((counts + P - 1) // P))
    nT = NB * t_fix
    n_slots = nT * P

    starts = np.zeros(NC * NB + 1, dtype=np.int64)
    np.cumsum(counts, out=starts[1:])

    xe = np.zeros((NC, n_slots, DIM), np.float32)
    dstloc_sl = np.full((NC, n_slots), PAD_DST, np.float32)
    srcg_sl = np.zeros((NC, n_slots), np.int32)
    for c in range(NC):
        for b in range(NB):
            k = c * NB + b
            cnt = counts[k]
            if cnt == 0:
                continue
            s0 = starts[k]
            base = b * t_fix * P
            sl = slice(base, base + cnt)
            es = src_s[s0:s0 + cnt]
            xe[c, sl] = x[es]
            dstloc_sl[c, sl] = pos[dst_s[s0:s0 + cnt]]
            srcg_sl[c, sl] = rowv[es]
    # slot s lives at [partition s%128, tile s//128]
    dstloc = dstloc_sl.reshape(NC, nT, P).transpose(0, 2, 1).copy()   # [NC,P,nT]
    srcg = srcg_sl.reshape(NC, nT, P).transpose(0, 2, 1).copy()
    xeT = np.ascontiguousarray(xe.transpose(0, 2, 1))                 # [NC,DIM,n_slots]
    # transposed masks MT[d, e] per tile, laid out [P, nT*P]
    dl = dstloc_sl.reshape(NC, nT, P)                                 # [c, t, e]
    mtb = (dl[:, :, None, :] == np.arange(P, dtype=np.float32)[None, None, :, None])
    mtb = np.ascontiguousarray(
        mtb.astype(np.float32).transpose(0, 2, 1, 3).reshape(NC, P, nT * P))

    # own-block x transposed (relabeled rows), padded
    xall = np.zeros((NC * BLKP, DIM), np.float32)
    xall[rowv] = x
    xTo = np.zeros((NC, DIM, BLKP), np.float32)
    for c in range(NC):
        xTo[c] = xall[c * BLKP:(c + 1) * BLKP].T

    # pred edges sharded by position
    n_pred = p.shape[1] + n.shape[1]
    pp = n_pred // NC
    npt = (pp + P - 1) // P
    pa = np.zeros((NC, npt * P), np.int32)
    pb = np.zeros((NC, npt * P), np.int32)
    allp = np.concatenate([p, n], axis=1)
    for c in range(NC):
        seg = allp[:, c * pp:(c + 1) * pp]
        pa[c, :pp] = row_of(seg[0])
        pb[c, :pp] = row_of(seg[1])
    pa = pa.reshape(NC, npt, P).transpose(0, 2, 1).copy()
    pb = pb.reshape(NC, npt, P).transpose(0, 2, 1).copy()

    # weight prep (weights only)
    W1 = np.asarray(W1, np.float32)
    W2 = np.asarray(W2, np.float32)
    W1h = W1.reshape(HEADS, HID, DIM)
    was1 = np.einsum('kh,khd->kd', np.asarray(a_src1, np.float32), W1h)
    wad1 = np.einsum('kh,khd->kd', np.asarray(a_dst1, np.float32), W1h)
    W1T = np.ascontiguousarray(W1.T)                                   # [128, 1024]
    b1 = np.asarray(b1, np.float32)
    b1T = b1.reshape(8, P).T.copy()                                    # [128, 8] col c
    W2T = np.ascontiguousarray(W2.T)                                   # [1024, 128]
    was2 = W2T @ np.asarray(a_src2, np.float32)[0]                     # [1024]
    wad2 = W2T @ np.asarray(a_dst2, np.float32)[0]
    W2aug = np.zeros((8, P, 130), np.float32)
    for c in range(8):
        W2aug[c, :, :128] = W2T[c * P:(c + 1) * P]
        W2aug[c, :, 128] = was2[c * P:(c + 1) * P]
        W2aug[c, :, 129] = wad2[c * P:(c + 1) * P]
    W2augP = np.ascontiguousarray(W2aug.transpose(1, 0, 2)).reshape(P, 8 * 130)
    b2_bc = np.tile(np.asarray(b2, np.float32)[None, :], (P, 1))       # [128, 128]
    clsWT = np.ascontiguousarray(np.asarray(cls_W, np.float32).T)      # [128, 4]
    clsb_bc = np.tile(np.asarray(cls_b, np.float32)[None, :], (P, 1))  # [128, 4]
    iota_f = np.tile(np.arange(P, dtype=np.float32)[None, :], (P, 1))
    ident = np.eye(P, dtype=np.float32)

    shared = dict(W1T=W1T, wasT1=np.ascontiguousarray(was1.T),
                  wadT1=np.ascontiguousarray(wad1.T), b1T=b1T,
                  W2augP=W2augP, b2_bc=b2_bc, clsWT=clsWT, clsb_bc=clsb_bc,
                  iota_f=iota_f, ident=ident)
    per_core = []
    for c in range(NC):
        m = dict(shared)
        m.update(xe=xe[c], xeT=xeT[c], xTo=xTo[c], dstloc=dstloc[c],
                 srcg=srcg[c], pa=pa[c], pb=pb[c], mtb=mtb[c])
        per_core.append(m)
    return per_core, t_fix, npt, pp, rowv


def _build(t_fix, npt):
    import concourse.bacc as bacc
    import concourse.mybir as mybir
    import concourse.tile as tile
    from concourse.bass import IndirectOffsetOnAxis

    F32 = mybir.dt.float32
    AF = mybir.ActivationFunctionType
    OP = mybir.AluOpType
    T = t_fix
    nT = NB * T
    n_slots = nT * P

    nc = bacc.Bacc("TRN2", target_bir_lowering=False, debug=False, num_devices=NC)
    din = {}
    for name, shape, dt in [
        ("xe", [n_slots, DIM], F32), ("xeT", [DIM, n_slots], F32),
        ("mtb", [P, n_slots], F32),
        ("xTo", [DIM, BLKP], F32), ("dstloc", [P, nT], F32),
        ("srcg", [P, nT], mybir.dt.int32),
        ("pa", [P, npt], mybir.dt.int32), ("pb", [P, npt], mybir.dt.int32),
        ("W1T", [P, HEADS * HID], F32), ("wasT1", [P, HEADS], F32),
        ("wadT1", [P, HEADS], F32), ("b1T", [P, 8], F32),
        ("W2augP", [P, 8 * 130], F32), ("b2_bc", [P, EMB], F32),
        ("clsWT", [P, 4], F32), ("clsb_bc", [P, 4], F32),
        ("iota_f", [P, P], F32), ("ident", [P, P], F32),
    ]:
        din[name] = nc.dram_tensor(name, shape, dt, kind="ExternalInput")
    z_ext = nc.dram_tensor("z_out", [BLKP, EMB], F32, kind="ExternalOutput")
    lg_ext = nc.dram_tensor("lg_out", [P, NB * 4], F32, kind="ExternalOutput")
    pr_ext = nc.dram_tensor("pr_out", [P, npt], F32, kind="ExternalOutput")

    g_own = nc.dram_tensor("g_own", [BLKP, GW], F32)
    g_all = nc.dram_tensor("g_all", [NC * BLKP, GW], F32, addr_space="Shared")
    z_own = nc.dram_tensor("z_own", [BLKP, EMB], F32)
    z_all = nc.dram_tensor("z_all", [NC * BLKP, EMB], F32, addr_space="Shared")

    with tile.TileContext(nc) as tc:
        with tc.tile_pool(name="const", bufs=1) as cp:
            W1T = cp.tile_from(din["W1T"].ap())
            wasT1 = cp.tile_from(din["wasT1"].ap())
            wadT1 = cp.tile_from(din["wadT1"].ap())
            b1T = cp.tile_from(din["b1T"].ap())
            W2augP = cp.tile_from(din["W2augP"].ap())
            b2_bc = cp.tile_from(din["b2_bc"].ap())
            clsWT = cp.tile_from(din["clsWT"].ap())
            clsb_bc = cp.tile_from(din["clsb_bc"].ap())
            iota_f = cp.tile_from(din["iota_f"].ap())
            ident = cp.tile_from(din["ident"].ap())
            dstloc = cp.tile_from(din["dstloc"].ap())
            srcg = cp.tile_from(din["srcg"].ap())
            pa_t = cp.tile_from(din["pa"].ap())
            pb_t = cp.tile_from(din["pb"].ap())
            zl_s = cp.tile([1, P], F32)
            nc.vector.memset(zl_s[:], 0.0)
            zr_s = cp.tile([1, 512], F32)
            nc.vector.memset(zr_s[:], 0.0)
            ad2_sb = cp.tile([P, NB], F32)
            lg_acc = cp.tile([P, NB * 4], F32)
            pr_acc = cp.tile([P, npt], F32)

            # ---------------- L1 + g-table ----------------
            with tc.tile_pool(name="sb1", bufs=2) as sb, \
                 tc.tile_pool(name="sbm", bufs=3) as sbm, \
                 tc.tile_pool(name="ps_acc", bufs=2, space="PSUM") as ps_acc, \
                 tc.tile_pool(name="ps_tr", bufs=2, space="PSUM") as ps_tr, \
                 tc.tile_pool(name="ps_o", bufs=1, space="PSUM") as ps_o:
                for b in range(NB):
                    e0 = b * T * P
                    xe_b = sb.tile([P, T * DIM], F32, tag="xe")
                    nc.sync.dma_start(
                        xe_b[:].rearrange("p (t d) -> p t d", t=T),
                        din["xe"][e0:e0 + T * P, :].rearrange("(t p) d -> p t d", p=P))
                    xeT_b = sb.tile([P, T * P], F32, tag="xeT")
                    nc.sync.dma_start(xeT_b[:], din["xeT"][:, e0:e0 + T * P])
                    mt_b = sb.tile([P, T * P], F32, tag="mt")
                    nc.sync.dma_start(mt_b[:], din["mtb"][:, e0:e0 + T * P])
                    xTo_t = sbm.tile([P, P], F32, tag="xTo")
                    nc.sync.dma_start(xTo_t[:], din["xTo"][:, b * P:(b + 1) * P])

                    ad1_ps = ps_tr.tile([P, HEADS], F32, tag="asad", padded_shape=[P, 512])
                    nc.tensor.matmul(ad1_ps[:], lhsT=xTo_t[:], rhs=wadT1[:],
                                     start=True, stop=True)
                    ad1_s = sbm.tile([P, HEADS], F32, tag="ad1s")
                    nc.vector.tensor_copy(ad1_s[:], ad1_ps[:])

                    # batched as+ad for all T tiles: [e, 4*T]
                    asad_ps = ps_tr.tile([P, 4 * T], F32, tag="asad", padded_shape=[P, 512])
                    nc.tensor.matmul(asad_ps[:], lhsT=zl_s[:], rhs=zr_s[:, 0:4 * T],
                                     start=True, stop=False)
                    for j in range(T):
                        nc.tensor.matmul(asad_ps[:, 4 * j:4 * j + 4],
                                         lhsT=xeT_b[:, j * P:(j + 1) * P],
                                         rhs=wasT1[:], start=False, stop=False)
                        nc.tensor.matmul(asad_ps[:, 4 * j:4 * j + 4],
                                         lhsT=mt_b[:, j * P:(j + 1) * P],
                                         rhs=ad1_s[:], start=False,
                                         stop=(j == T - 1))
                    t02 = sbm.tile([P, 4 * T], F32, tag="t02")
                    nc.vector.tensor_scalar(out=t02[:], in0=asad_ps[:], scalar1=SLOPE,
                                            scalar2=None, op0=OP.mult)
                    e_s = sbm.tile([P, 4 * T], F32, tag="e")
                    nc.vector.tensor_tensor(out=e_s[:], in0=asad_ps[:], in1=t02[:],
                                            op=OP.max)
                    al_s = sbm.tile([P, 4 * T], F32, tag="al")
                    nc.scalar.activation(al_s[:], e_s[:], AF.Exp)

                    agg01 = ps_acc.tile([P, 258], F32, tag="agg01", padded_shape=[P, 512])
                    agg23 = ps_acc.tile([P, 258], F32, tag="agg23", padded_shape=[P, 512])
                    nc.tensor.matmul(agg01[:], lhsT=zl_s[:], rhs=zr_s[:, 0:258],
                                     start=True, stop=False)
                    nc.tensor.matmul(agg23[:], lhsT=zl_s[:], rhs=zr_s[:, 0:258],
                                     start=True, stop=False)
                    for j in range(T):
                        m_s = sbm.tile([P, P], F32, tag="m")
                        nc.vector.tensor_tensor(
                            out=m_s[:],
                            in0=dstloc[:, b * T + j:b * T + j + 1].to_broadcast([P, P]),
                            in1=iota_f[:], op=OP.is_equal)
                        xw01 = sbm.tile([P, 258], F32, tag="xw01")
                        xw23 = sbm.tile([P, 258], F32, tag="xw23")
                        for k in range(4):
                            dstt = (xw01, xw23)[k // 2]
                            nc.vector.tensor_scalar(
                                out=dstt[:, (k % 2) * P:(k % 2) * P + P],
                                in0=xe_b[:, j * DIM:(j + 1) * DIM],
                                scalar1=al_s[:, 4 * j + k:4 * j + k + 1],
                                scalar2=None, op0=OP.mult)
                        nc.vector.tensor_copy(xw01[:, 256:258],
                                              al_s[:, 4 * j:4 * j + 2])
                        nc.vector.tensor_copy(xw23[:, 256:258],
                                              al_s[:, 4 * j + 2:4 * j + 4])
                        last = (j == T - 1)
                        nc.tensor.matmul(agg01[:], lhsT=m_s[:], rhs=xw01[:],
                                         start=False, stop=last)
                        nc.tensor.matmul(agg23[:], lhsT=m_s[:], rhs=xw23[:],
                                         start=False, stop=last)

                    # block tail
                    denr = sbm.tile([P, HEADS], F32, tag="denr")
                    nc.vector.tensor_scalar(out=denr[:, 0:2], in0=agg01[:, 256:258],
                                            scalar1=1e-16, scalar2=None, op0=OP.add)
                    nc.vector.tensor_scalar(out=denr[:, 2:4], in0=agg23[:, 256:258],
                                            scalar1=1e-16, scalar2=None, op0=OP.add)
                    nc.vector.reciprocal(denr[:], denr[:])
                    aggn = sb.tile([P, 4 * P], F32, tag="aggn")
                    for k in range(4):
                        srct = (agg01, agg23)[k // 2]
                        nc.vector.tensor_scalar(
                            out=aggn[:, k * P:(k + 1) * P],
                            in0=srct[:, (k % 2) * P:(k % 2) * P + P],
                            scalar1=denr[:, k:k + 1], scalar2=None, op0=OP.mult)
                    aggT_ps = ps_o.tile([P, 4 * P], F32, tag="tail", padded_shape=[P, 1024])
                    for k in range(4):
                        nc.tensor.transpose(aggT_ps[:, k * P:(k + 1) * P],
                                            aggn[:, k * P:(k + 1) * P], ident[:])
                    aggT_s = sb.tile([P, 4 * P], F32, tag="aggTs")
                    nc.vector.tensor_copy(aggT_s[:], aggT_ps[:])

                    o1_ps = ps_o.tile([P, 8 * P], F32, tag="tail", padded_shape=[P, 1024])
                    for c in range(8):
                        nc.tensor.matmul(
                            o1_ps[:, c * P:(c + 1) * P],
                            lhsT=W1T[:, c * P:(c + 1) * P],
                            rhs=aggT_s[:, (c // 2) * P:(c // 2 + 1) * P],
                            start=True, stop=True)
                    h1T = sb.tile([P, 8 * P], F32, tag="h1T")
                    for c in range(8):
                        tt = sbm.tile([P, P], F32, tag="elu_t")
                        rr = sbm.tile([P, P], F32, tag="elu_r")
                        nc.scalar.activation(tt[:], o1_ps[:, c * P:(c + 1) * P],
                                             AF.Exp, bias=b1T[:, c:c + 1])
                        nc.scalar.activation(rr[:], o1_ps[:, c * P:(c + 1) * P],
                                             AF.Relu, bias=b1T[:, c:c + 1])
                        nc.vector.tensor_scalar(out=tt[:], in0=tt[:], scalar1=-1.0,
                                                scalar2=None, op0=OP.add)
                        nc.vector.tensor_tensor(out=h1T[:, c * P:(c + 1) * P],
                                                in0=tt[:], in1=rr[:], op=OP.min)
                    gaug_ps = ps_o.tile([P, 130], F32, tag="tail", padded_shape=[P, 1024])
                    for c in range(8):
                        nc.tensor.matmul(gaug_ps[:], lhsT=h1T[:, c * P:(c + 1) * P],
                                         rhs=W2augP[:, c * 130:(c + 1) * 130],
                                         start=(c == 0), stop=(c == 7))
                    g_sb = sb.tile([P, GW], F32, tag="gsb")
                    nc.vector.tensor_copy(g_sb[:, 0:128], gaug_ps[:, 0:128])
                    nc.vector.memset(g_sb[:, 128:129], 1.0)
                    nc.vector.tensor_copy(g_sb[:, 129:131], gaug_ps[:, 128:130])
                    nc.vector.tensor_copy(ad2_sb[:, b:b + 1], gaug_ps[:, 129:130])
                    nc.sync.dma_start(g_own[b * P:(b + 1) * P, :], g_sb[:])

            # ---------------- AllGather g ----------------
            nc.gpsimd.collective_compute(
                "AllGather", mybir.AluOpType.bypass,
                replica_groups=[list(range(NC))],
                ins=[g_own.ap().opt()], outs=[g_all.ap().opt()])

            # ---------------- L2 ----------------
            with tc.tile_pool(name="sb2", bufs=2) as sb, \
                 tc.tile_pool(name="sbg", bufs=2 * T + 2) as sbg, \
                 tc.tile_pool(name="sbm2", bufs=3) as sbm, \
                 tc.tile_pool(name="ps2_acc", bufs=2, space="PSUM") as ps_acc, \
                 tc.tile_pool(name="ps2_tr", bufs=2, space="PSUM") as ps_tr:
                for b in range(NB):
                    mt_b = sb.tile([P, T * P], F32, tag="mt2")
                    e0 = b * T * P
                    nc.sync.dma_start(mt_b[:], din["mtb"][:, e0:e0 + T * P])
                    g_ts = []
                    as2_all = sbm.tile([P, T], F32, tag="as2a")
                    e2_ps = ps_tr.tile([P, T], F32, tag="e2", padded_shape=[P, 512])
                    nc.tensor.matmul(e2_ps[:], lhsT=zl_s[:], rhs=zr_s[:, 0:T],
                                     start=True, stop=False)
                    for j in range(T):
                        g_t = sbg.tile([P, GW], F32, tag="gt")
                        nc.gpsimd.indirect_dma_start(
                            out=g_t[:], out_offset=None, in_=g_all[:, :],
                            in_offset=IndirectOffsetOnAxis(
                                ap=srcg[:, b * T + j:b * T + j + 1], axis=0))
                        g_ts.append(g_t)
                        nc.vector.tensor_copy(as2_all[:, j:j + 1], g_t[:, 129:130])
                        nc.tensor.matmul(e2_ps[:, j:j + 1],
                                         lhsT=mt_b[:, j * P:(j + 1) * P],
                                         rhs=ad2_sb[:, b:b + 1], start=False,
                                         stop=(j == T - 1))
                    s2_s = sbm.tile([P, T], F32, tag="s2s")
                    nc.vector.tensor_tensor(out=s2_s[:], in0=e2_ps[:], in1=as2_all[:],
                                            op=OP.add)
                    t2_s = sbm.tile([P, T], F32, tag="t2s")
                    nc.vector.tensor_scalar(out=t2_s[:], in0=s2_s[:], scalar1=SLOPE,
                                            scalar2=None, op0=OP.mult)
                    e2m = sbm.tile([P, T], F32, tag="e2m")
                    nc.vector.tensor_tensor(out=e2m[:], in0=s2_s[:], in1=t2_s[:],
                                            op=OP.max)
                    al2 = sbm.tile([P, T], F32, tag="al2")
                    nc.scalar.activation(al2[:], e2m[:], AF.Exp)

                    agg2_ps = ps_acc.tile([P, 129], F32, tag="agg2", padded_shape=[P, 512])
                    nc.tensor.matmul(agg2_ps[:], lhsT=zl_s[:], rhs=zr_s[:, 0:129],
                                     start=True, stop=False)
                    for j in range(T):
                        m_s = sbm.tile([P, P], F32, tag="m2")
                        nc.vector.tensor_tensor(
                            out=m_s[:],
                            in0=dstloc[:, b * T + j:b * T + j + 1].to_broadcast([P, P]),
                            in1=iota_f[:], op=OP.is_equal)
                        rhs2 = sbm.tile([P, 129], F32, tag="rhs2")
                        nc.vector.tensor_scalar(out=rhs2[:], in0=g_ts[j][:, 0:129],
                                                scalar1=al2[:, j:j + 1], scalar2=None,
                                                op0=OP.mult)
                        nc.tensor.matmul(agg2_ps[:], lhsT=m_s[:], rhs=rhs2[:],
                                         start=False, stop=(j == T - 1))
                    den2r = sbm.tile([P, 1], F32, tag="den2r")
                    nc.vector.tensor_scalar(out=den2r[:], in0=agg2_ps[:, 128:129],
                                            scalar1=1e-16, scalar2=None, op0=OP.add)
                    nc.vector.reciprocal(den2r[:], den2r[:])
                    y_s = sb.tile([P, EMB], F32, tag="y")
                    nc.vector.tensor_scalar(out=y_s[:], in0=agg2_ps[:, 0:128],
                                            scalar1=den2r[:, 0:1], scalar2=None,
                                            op0=OP.mult)
                    nc.vector.tensor_tensor(out=y_s[:], in0=y_s[:], in1=b2_bc[:],
                                            op=OP.add)
                    tt = sbm.tile([P, EMB], F32, tag="z_t")
                    rr = sbm.tile([P, EMB], F32, tag="z_r")
                    nc.scalar.activation(tt[:], y_s[:], AF.Exp)
                    nc.scalar.activation(rr[:], y_s[:], AF.Relu)
                    nc.vector.tensor_scalar(out=tt[:], in0=tt[:], scalar1=-1.0,
                                            scalar2=None, op0=OP.add)
                    z_sb = sb.tile([P, EMB], F32, tag="zsb")
                    nc.vector.tensor_tensor(out=z_sb[:], in0=tt[:], in1=rr[:], op=OP.min)
                    nc.sync.dma_start(z_own[b * P:(b + 1) * P, :], z_sb[:])
                    nc.sync.dma_start(z_ext[b * P:(b + 1) * P, :], z_sb[:])
                    zT_ps = ps_tr.tile([P, P], F32, tag="ztail", padded_shape=[P, 512])
                    nc.tensor.transpose(zT_ps[:], z_sb[:], ident[:])
                    zT_s = sbm.tile([P, P], F32, tag="zTs")
                    nc.vector.tensor_copy(zT_s[:], zT_ps[:])
                    lg_ps = ps_tr.tile([P, 4], F32, tag="ztail", padded_shape=[P, 512])
                    nc.tensor.matmul(lg_ps[:], lhsT=zT_s[:], rhs=clsWT[:],
                                     start=True, stop=True)
                    nc.vector.tensor_tensor(out=lg_acc[:, b * 4:(b + 1) * 4],
                                            in0=lg_ps[:], in1=clsb_bc[:], op=OP.add)

            # ---------------- AllGather z, preds ----------------
            nc.gpsimd.collective_compute(
                "AllGather", mybir.AluOpType.bypass,
                replica_groups=[list(range(NC))],
                ins=[z_own.ap().opt()], outs=[z_all.ap().opt()])

            with tc.tile_pool(name="sb3", bufs=4) as sb:
                sums = cp.tile([P, npt], F32)
                for q in range(npt):
                    za = sb.tile([P, EMB], F32, tag="za")
                    nc.gpsimd.indirect_dma_start(
                        out=za[:], out_offset=None, in_=z_all[:, :],
                        in_offset=IndirectOffsetOnAxis(ap=pa_t[:, q:q + 1], axis=0))
                    zb = sb.tile([P, EMB], F32, tag="zb")
                    nc.gpsimd.indirect_dma_start(
                        out=zb[:], out_offset=None, in_=z_all[:, :],
                        in_offset=IndirectOffsetOnAxis(ap=pb_t[:, q:q + 1], axis=0))
                    mm = sb.tile([P, EMB], F32, tag="zm")
                    nc.vector.tensor_tensor(out=mm[:], in0=za[:], in1=zb[:],
                                            op=OP.mult)
                    nc.vector.reduce_sum(out=sums[:, q:q + 1], in_=mm[:],
                                         axis=mybir.AxisListType.X)
                nc.scalar.activation(pr_acc[:], sums[:], AF.Sigmoid)
            nc.sync.dma_start(lg_ext[:, :], lg_acc[:])
            nc.sync.dma_start(pr_ext[:, :], pr_acc[:])
    nc.compile()
    return nc


_LAST_EXEC_NS = None


def _trace_kwargs():
    import os
    if not os.environ.get("GAT_TRACE"):
        return {}
    import sys, types
    try:
        import antenv.axon_hooks  # noqa: F401
    except ImportError:
        import antenv
        mod = types.ModuleType("antenv.axon_hooks")
        mod._hook = None
        mod.set_axon_ntff_profile_hook = lambda h: setattr(mod, "_hook", h)
        mod.get_axon_ntff_profile_hook = lambda: mod._hook
        sys.modules["antenv.axon_hooks"] = mod
        antenv.axon_hooks = mod
        try:
            from trn_agent_boot.trn_boot import _ntff_profile_via_ctypes
            mod.set_axon_ntff_profile_hook(
                _ntff_profile_via_ctypes("/opt/axon/libaxon_pjrt.so"))
        except Exception:
            return {}
    return {"trace": True}


def kernel(**inputs):
    global _LAST_EXEC_NS
    from concourse.bass_utils import run_bass_kernel_spmd
    per_core, t_fix, npt, pp, rowv = _prep(**inputs)
    nc = _build(t_fix, npt)
    res = run_bass_kernel_spmd(nc, per_core, core_ids=list(range(NC)),
                               **_trace_kwargs())
    _LAST_EXEC_NS = res.exec_time_ns
    z_cat = np.concatenate([res.results[c]["z_out"] for c in range(NC)], axis=0)
    lg_cat = np.concatenate(
        [res.results[c]["lg_out"].reshape(P, NB, 4).transpose(1, 0, 2)
         .reshape(BLKP, 4) for c in range(NC)], axis=0)
    z = z_cat[rowv]
    logits = lg_cat[rowv]
    preds = np.concatenate(
        [res.results[c]["pr_out"].T.reshape(npt * P)[:pp] for c in range(NC)])
    return z.astype(np.float32), logits.astype(np.float32), preds.astype(np.float32)


# revision 17
# speedup vs baseline: 1.0836x; 1.0448x over previous
"""2-layer GAT (GATConv x2 + link predictor) on 8 Trainium2 NeuronCores.

Sharding: nodes partitioned into 8 contiguous blocks (graph parallel).
Each core aggregates incoming edges of its own destination block.
L1 source features are host-pre-gathered into edge-slot order (x is an
input); L2 aggregates the device-computed g-table (AllGather'd across
cores) with device-side indirect-DMA gathers, and link predictions
gather rows of the AllGather'd z-table.

Segment softmax / segment sum are done with per-tile mask matmuls on the
tensor engine: edges are sorted by destination and padded per 128-node
destination tile; M[e,d] = (dstloc[e]==d) maps 128 edges onto the tile's
nodes, so M.T @ (alpha * feat) accumulates per-node sums in PSUM.
"""
import numpy as np

# model dims (from the reference problem; fixed by the harness)
N_NODES = 50000
DIM = 128
HEADS = 4
HID = 256
EMB = 128
SLOPE = 0.2

NC = 8
P = 128
NB = 50                        # dst tiles (node groups) per core
GRP = N_NODES // (NC * NB)     # 125 real nodes per group (3 pad slots)
BLKP = NB * P                  # 6400 padded rows per core
GW = 131                       # g-table row: [g(128) | 1.0 | as2 | ad2]
PAD_DST = 999.0


def _prep(x, e, p, n, W1, a_src1, a_dst1, b1, W2, a_src2, a_dst2, b2, cls_W, cls_b):
    """Host-side graph/index/layout prep. No model FLOPs on node data."""
    x = np.asarray(x, np.float32)
    e = np.asarray(e, np.int64)
    p = np.asarray(p, np.int64)
    n = np.asarray(n, np.int64)

    loop = np.arange(N_NODES, dtype=np.int64)
    src = np.concatenate([e[0], loop])
    dst = np.concatenate([e[1], loop])

    # degree-balanced node grouping: snake-assign nodes (sorted by in-degree)
    # to NC*NB groups of GRP nodes so per-group edge counts are uniform.
    deg = np.bincount(dst, minlength=N_NODES)
    by_deg = np.argsort(-deg, kind="stable")
    ngrp = NC * NB
    grp = np.zeros(N_NODES, np.int64)
    pos = np.zeros(N_NODES, np.int64)
    for pss in range(GRP):
        seg = by_deg[pss * ngrp:(pss + 1) * ngrp]
        gids = np.arange(ngrp) if pss % 2 == 0 else np.arange(ngrp)[::-1]
        grp[seg] = gids
        pos[seg] = pss
    rowv = grp * P + pos                      # node -> padded table row
    row_of = lambda g: rowv[g]

    # per (core, dst-tile) edge lists
    key = grp[dst]
    core_of = key // NB
    order = np.lexsort((dst, key))
    src_s, dst_s = src[order], dst[order]
    counts = np.bincount(key, minlength=NC * NB)
    t_fix = int(np.max((counts + P - 1) // P))
    nT = NB * t_fix
    n_slots = nT * P

    starts = np.zeros(NC * NB + 1, dtype=np.int64)
    np.cumsum(counts, out=starts[1:])

    xe = np.zeros((NC, n_slots, DIM), np.float32)
    dstloc_sl = np.full((NC, n_slots), PAD_DST, np.float32)
    srcg_sl = np.zeros((NC, n_slots), np.int32)
    for c in range(NC):
        for b in range(NB):
            k = c * NB + b
            cnt = counts[k]
            if cnt == 0:
                continue
            s0 = starts[k]
            base = b * t_fix * P
            sl = slice(base, base + cnt)
            es = src_s[s0:s0 + cnt]
            xe[c, sl] = x[es]
            dstloc_sl[c, sl] = pos[dst_s[s0:s0 + cnt]]
            srcg_sl[c, sl] = rowv[es]
    # slot s lives at [partition s%128, tile s//128]
    dstloc = dstloc_sl.reshape(NC, nT, P).transpose(0, 2, 1).copy()   # [NC,P,nT]
    srcg = srcg_sl.reshape(NC, nT, P).transpose(0, 2, 1).copy()
    xeT = np.ascontiguousarray(xe.transpose(0, 2, 1).astype(np.float16))  # [NC,DIM,n_slots]
    # transposed masks MT[d, e] per tile, laid out [P, nT*P]
    dl = dstloc_sl.reshape(NC, nT, P)                                 # [c, t, e]
    mtb = (dl[:, :, None, :] == np.arange(P, dtype=np.float32)[None, None, :, None])
    mtb = np.ascontiguousarray(
        mtb.astype(np.float16).transpose(0, 2, 1, 3).reshape(NC, P, nT * P))

    # own-block x transposed (relabeled rows), padded
    xall = np.zeros((NC * BLKP, DIM), np.float32)
    xall[rowv] = x
    xTo = np.zeros((NC, DIM, BLKP), np.float32)
    for c in range(NC):
        xTo[c] = xall[c * BLKP:(c + 1) * BLKP].T

    # pred edges sharded by position
    n_pred = p.shape[1] + n.shape[1]
    pp = n_pred // NC
    npt = (pp + P - 1) // P
    pa = np.zeros((NC, npt * P), np.int32)
    pb = np.zeros((NC, npt * P), np.int32)
    allp = np.concatenate([p, n], axis=1)
    for c in range(NC):
        seg = allp[:, c * pp:(c + 1) * pp]
        pa[c, :pp] = row_of(seg[0])
        pb[c, :pp] = row_of(seg[1])
    pa = pa.reshape(NC, npt, P).transpose(0, 2, 1).copy()
    pb = pb.reshape(NC, npt, P).transpose(0, 2, 1).copy()

    # weight prep (weights only)
    W1 = np.asarray(W1, np.float32)
    W2 = np.asarray(W2, np.float32)
    W1h = W1.reshape(HEADS, HID, DIM)
    was1 = np.einsum('kh,khd->kd', np.asarray(a_src1, np.float32), W1h)
    wad1 = np.einsum('kh,khd->kd', np.asarray(a_dst1, np.float32), W1h)
    W1T = np.ascontiguousarray(W1.T)                                   # [128, 1024]
    b1 = np.asarray(b1, np.float32)
    b1T = b1.reshape(8, P).T.copy()                                    # [128, 8] col c
    W2T = np.ascontiguousarray(W2.T)                                   # [1024, 128]
    was2 = W2T @ np.asarray(a_src2, np.float32)[0]                     # [1024]
    wad2 = W2T @ np.asarray(a_dst2, np.float32)[0]
    W2aug = np.zeros((8, P, 130), np.float32)
    for c in range(8):
        W2aug[c, :, :128] = W2T[c * P:(c + 1) * P]
        W2aug[c, :, 128] = was2[c * P:(c + 1) * P]
        W2aug[c, :, 129] = wad2[c * P:(c + 1) * P]
    W2augP = np.ascontiguousarray(W2aug.transpose(1, 0, 2)).reshape(P, 8 * 130)
    b2_bc = np.tile(np.asarray(b2, np.float32)[None, :], (P, 1))       # [128, 128]
    clsWT = np.ascontiguousarray(np.asarray(cls_W, np.float32).T)      # [128, 4]
    clsb_bc = np.tile(np.asarray(cls_b, np.float32)[None, :], (P, 1))  # [128, 4]
    iota_f = np.tile(np.arange(P, dtype=np.float32)[None, :], (P, 1))
    ident = np.eye(P, dtype=np.float32)

    shared = dict(W1T=W1T, wasT1=np.ascontiguousarray(was1.T.astype(np.float16)),
                  wadT1=np.ascontiguousarray(wad1.T), b1T=b1T,
                  W2augP=W2augP, b2_bc=b2_bc, clsWT=clsWT, clsb_bc=clsb_bc,
                  iota_f=iota_f, ident=ident)
    per_core = []
    for c in range(NC):
        m = dict(shared)
        m.update(xe=xe[c], xeT=xeT[c], xTo=xTo[c], dstloc=dstloc[c],
                 srcg=srcg[c], pa=pa[c], pb=pb[c], mtb=mtb[c])
        per_core.append(m)
    return per_core, t_fix, npt, pp, rowv


def _build(t_fix, npt):
    import concourse.bacc as bacc
    import concourse.mybir as mybir
    import concourse.tile as tile
    from concourse.bass import IndirectOffsetOnAxis

    F32 = mybir.dt.float32
    F16 = mybir.dt.float16
    AF = mybir.ActivationFunctionType
    OP = mybir.AluOpType
    T = t_fix
    nT = NB * T
    n_slots = nT * P

    nc = bacc.Bacc("TRN2", target_bir_lowering=False, debug=False, num_devices=NC)
    din = {}
    for name, shape, dt in [
        ("xe", [n_slots, DIM], F32), ("xeT", [DIM, n_slots], F16),
        ("mtb", [P, n_slots], F16),
        ("xTo", [DIM, BLKP], F32), ("dstloc", [P, nT], F32),
        ("srcg", [P, nT], mybir.dt.int32),
        ("pa", [P, npt], mybir.dt.int32), ("pb", [P, npt], mybir.dt.int32),
        ("W1T", [P, HEADS * HID], F32), ("wasT1", [P, HEADS], F16),
        ("wadT1", [P, HEADS], F32), ("b1T", [P, 8], F32),
        ("W2augP", [P, 8 * 130], F32), ("b2_bc", [P, EMB], F32),
        ("clsWT", [P, 4], F32), ("clsb_bc", [P, 4], F32),
        ("iota_f", [P, P], F32), ("ident", [P, P], F32),
    ]:
        din[name] = nc.dram_tensor(name, shape, dt, kind="ExternalInput")
    z_ext = nc.dram_tensor("z_out", [BLKP, EMB], F32, kind="ExternalOutput")
    lg_ext = nc.dram_tensor("lg_out", [P, NB * 4], F32, kind="ExternalOutput")
    pr_ext = nc.dram_tensor("pr_out", [P, npt], F32, kind="ExternalOutput")

    g_own = nc.dram_tensor("g_own", [BLKP, GW], F32)
    g_all = nc.dram_tensor("g_all", [NC * BLKP, GW], F32, addr_space="Shared")
    z_own = nc.dram_tensor("z_own", [BLKP, EMB], F32)
    z_all = nc.dram_tensor("z_all", [NC * BLKP, EMB], F32, addr_space="Shared")

    with tile.TileContext(nc) as tc:
        with tc.tile_pool(name="const", bufs=1) as cp:
            W1T = cp.tile_from(din["W1T"].ap())
            wasT1 = cp.tile_from(din["wasT1"].ap())
            wadT1 = cp.tile_from(din["wadT1"].ap())
            b1T = cp.tile_from(din["b1T"].ap())
            W2augP = cp.tile_from(din["W2augP"].ap())
            b2_bc = cp.tile_from(din["b2_bc"].ap())
            clsWT = cp.tile_from(din["clsWT"].ap())
            clsb_bc = cp.tile_from(din["clsb_bc"].ap())
            iota_f = cp.tile_from(din["iota_f"].ap())
            ident = cp.tile_from(din["ident"].ap())
            dstloc = cp.tile_from(din["dstloc"].ap())
            srcg = cp.tile_from(din["srcg"].ap())
            pa_t = cp.tile_from(din["pa"].ap())
            pb_t = cp.tile_from(din["pb"].ap())
            zl_s = cp.tile([1, P], F32)
            nc.vector.memset(zl_s[:], 0.0)
            zr_s = cp.tile([1, 512], F32)
            nc.vector.memset(zr_s[:], 0.0)
            ad2_sb = cp.tile([P, NB], F32)
            lg_acc = cp.tile([P, NB * 4], F32)
            pr_acc = cp.tile([P, npt], F32)

            # ---------------- L1 + g-table ----------------
            with tc.tile_pool(name="sb1", bufs=2) as sb, \
                 tc.tile_pool(name="sbm", bufs=3) as sbm, \
                 tc.tile_pool(name="ps_acc", bufs=2, space="PSUM") as ps_acc, \
                 tc.tile_pool(name="ps_tr", bufs=2, space="PSUM") as ps_tr, \
                 tc.tile_pool(name="ps_o", bufs=1, space="PSUM") as ps_o:
                for b in range(NB):
                    e0 = b * T * P
                    xe_b = sb.tile([P, T * DIM], F32, tag="xe")
                    nc.sync.dma_start(
                        xe_b[:].rearrange("p (t d) -> p t d", t=T),
                        din["xe"][e0:e0 + T * P, :].rearrange("(t p) d -> p t d", p=P))
                    xeT_b = sb.tile([P, T * P], F16, tag="xeT")
                    nc.sync.dma_start(xeT_b[:], din["xeT"][:, e0:e0 + T * P])
                    mt_b = sb.tile([P, T * P], F16, tag="mt")
                    nc.sync.dma_start(mt_b[:], din["mtb"][:, e0:e0 + T * P])
                    xTo_t = sbm.tile([P, P], F32, tag="xTo")
                    nc.sync.dma_start(xTo_t[:], din["xTo"][:, b * P:(b + 1) * P])

                    ad1_ps = ps_tr.tile([P, HEADS], F32, tag="asad", padded_shape=[P, 512])
                    nc.tensor.matmul(ad1_ps[:], lhsT=xTo_t[:], rhs=wadT1[:],
                                     start=True, stop=True)
                    ad1_s = sbm.tile([P, HEADS], F16, tag="ad1s")
                    nc.vector.tensor_copy(ad1_s[:], ad1_ps[:])

                    # batched as+ad for all T tiles: [e, 4*T]
                    asad_ps = ps_tr.tile([P, 4 * T], F32, tag="asad", padded_shape=[P, 512])
                    nc.tensor.matmul(asad_ps[:], lhsT=zl_s[:], rhs=zr_s[:, 0:4 * T],
                                     start=True, stop=False)
                    for j in range(T):
                        nc.tensor.matmul(asad_ps[:, 4 * j:4 * j + 4],
                                         lhsT=xeT_b[:, j * P:(j + 1) * P],
                                         rhs=wasT1[:], start=False, stop=False)
                        nc.tensor.matmul(asad_ps[:, 4 * j:4 * j + 4],
                                         lhsT=mt_b[:, j * P:(j + 1) * P],
                                         rhs=ad1_s[:], start=False,
                                         stop=(j == T - 1))
                    t02 = sbm.tile([P, 4 * T], F32, tag="t02")
                    nc.vector.tensor_scalar(out=t02[:], in0=asad_ps[:], scalar1=SLOPE,
                                            scalar2=None, op0=OP.mult)
                    e_s = sbm.tile([P, 4 * T], F32, tag="e")
                    nc.vector.tensor_tensor(out=e_s[:], in0=asad_ps[:], in1=t02[:],
                                            op=OP.max)
                    al_s = sbm.tile([P, 4 * T], F32, tag="al")
                    nc.scalar.activation(al_s[:], e_s[:], AF.Exp)

                    agg01 = ps_acc.tile([P, 258], F32, tag="agg01", padded_shape=[P, 512])
                    agg23 = ps_acc.tile([P, 258], F32, tag="agg23", padded_shape=[P, 512])
                    nc.tensor.matmul(agg01[:], lhsT=zl_s[:], rhs=zr_s[:, 0:258],
                                     start=True, stop=False)
                    nc.tensor.matmul(agg23[:], lhsT=zl_s[:], rhs=zr_s[:, 0:258],
                                     start=True, stop=False)
                    for j in range(T):
                        m_s = sbm.tile([P, P], F16, tag="m")
                        nc.vector.tensor_tensor(
                            out=m_s[:],
                            in0=dstloc[:, b * T + j:b * T + j + 1].to_broadcast([P, P]),
                            in1=iota_f[:], op=OP.is_equal)
                        xw01 = sbm.tile([P, 258], F16, tag="xw01")
                        xw23 = sbm.tile([P, 258], F16, tag="xw23")
                        for k in range(4):
                            dstt = (xw01, xw23)[k // 2]
                            nc.vector.tensor_scalar(
                                out=dstt[:, (k % 2) * P:(k % 2) * P + P],
                                in0=xe_b[:, j * DIM:(j + 1) * DIM],
                                scalar1=al_s[:, 4 * j + k:4 * j + k + 1],
                                scalar2=None, op0=OP.mult)
                        nc.vector.tensor_copy(xw01[:, 256:258],
                                              al_s[:, 4 * j:4 * j + 2])
                        nc.vector.tensor_copy(xw23[:, 256:258],
                                              al_s[:, 4 * j + 2:4 * j + 4])
                        last = (j == T - 1)
                        nc.tensor.matmul(agg01[:], lhsT=m_s[:], rhs=xw01[:],
                                         start=False, stop=last)
                        nc.tensor.matmul(agg23[:], lhsT=m_s[:], rhs=xw23[:],
                                         start=False, stop=last)

                    # block tail
                    denr = sbm.tile([P, HEADS], F32, tag="denr")
                    nc.vector.tensor_scalar(out=denr[:, 0:2], in0=agg01[:, 256:258],
                                            scalar1=1e-16, scalar2=None, op0=OP.add)
                    nc.vector.tensor_scalar(out=denr[:, 2:4], in0=agg23[:, 256:258],
                                            scalar1=1e-16, scalar2=None, op0=OP.add)
                    nc.vector.reciprocal(denr[:], denr[:])
                    aggn = sb.tile([P, 4 * P], F32, tag="aggn")
                    for k in range(4):
                        srct = (agg01, agg23)[k // 2]
                        nc.vector.tensor_scalar(
                            out=aggn[:, k * P:(k + 1) * P],
                            in0=srct[:, (k % 2) * P:(k % 2) * P + P],
                            scalar1=denr[:, k:k + 1], scalar2=None, op0=OP.mult)
                    aggT_ps = ps_o.tile([P, 4 * P], F32, tag="tail", padded_shape=[P, 1024])
                    for k in range(4):
                        nc.tensor.transpose(aggT_ps[:, k * P:(k + 1) * P],
                                            aggn[:, k * P:(k + 1) * P], ident[:])
                    aggT_s = sb.tile([P, 4 * P], F32, tag="aggTs")
                    nc.vector.tensor_copy(aggT_s[:], aggT_ps[:])

                    o1_ps = ps_o.tile([P, 8 * P], F32, tag="tail", padded_shape=[P, 1024])
                    for c in range(8):
                        nc.tensor.matmul(
                            o1_ps[:, c * P:(c + 1) * P],
                            lhsT=W1T[:, c * P:(c + 1) * P],
                            rhs=aggT_s[:, (c // 2) * P:(c // 2 + 1) * P],
                            start=True, stop=True)
                    h1T = sb.tile([P, 8 * P], F32, tag="h1T")
                    for c in range(8):
                        tt = sbm.tile([P, P], F32, tag="elu_t")
                        rr = sbm.tile([P, P], F32, tag="elu_r")
                        nc.scalar.activation(tt[:], o1_ps[:, c * P:(c + 1) * P],
                                             AF.Exp, bias=b1T[:, c:c + 1])
                        nc.scalar.activation(rr[:], o1_ps[:, c * P:(c + 1) * P],
                                             AF.Relu, bias=b1T[:, c:c + 1])
                        nc.vector.tensor_scalar(out=tt[:], in0=tt[:], scalar1=-1.0,
                                                scalar2=None, op0=OP.add)
                        nc.vector.tensor_tensor(out=h1T[:, c * P:(c + 1) * P],
                                                in0=tt[:], in1=rr[:], op=OP.min)
                    gaug_ps = ps_o.tile([P, 130], F32, tag="tail", padded_shape=[P, 1024])
                    for c in range(8):
                        nc.tensor.matmul(gaug_ps[:], lhsT=h1T[:, c * P:(c + 1) * P],
                                         rhs=W2augP[:, c * 130:(c + 1) * 130],
                                         start=(c == 0), stop=(c == 7))
                    g_sb = sb.tile([P, GW], F32, tag="gsb")
                    nc.vector.tensor_copy(g_sb[:, 0:128], gaug_ps[:, 0:128])
                    nc.vector.memset(g_sb[:, 128:129], 1.0)
                    nc.vector.tensor_copy(g_sb[:, 129:131], gaug_ps[:, 128:130])
                    nc.vector.tensor_copy(ad2_sb[:, b:b + 1], gaug_ps[:, 129:130])
                    nc.sync.dma_start(g_own[b * P:(b + 1) * P, :], g_sb[:])

            # ---------------- AllGather g ----------------
            nc.gpsimd.collective_compute(
                "AllGather", mybir.AluOpType.bypass,
                replica_groups=[list(range(NC))],
                ins=[g_own.ap().opt()], outs=[g_all.ap().opt()])

            # ---------------- L2 ----------------
            ad2_h = cp.tile([P, NB], F16)
            nc.vector.tensor_copy(ad2_h[:], ad2_sb[:])
            with tc.tile_pool(name="sb2", bufs=2) as sb, \
                 tc.tile_pool(name="sbg", bufs=2 * T + 2) as sbg, \
                 tc.tile_pool(name="sbm2", bufs=3) as sbm, \
                 tc.tile_pool(name="ps2_acc", bufs=2, space="PSUM") as ps_acc, \
                 tc.tile_pool(name="ps2_tr", bufs=2, space="PSUM") as ps_tr:
                for b in range(NB):
                    mt_b = sb.tile([P, T * P], F16, tag="mt2")
                    e0 = b * T * P
                    nc.sync.dma_start(mt_b[:], din["mtb"][:, e0:e0 + T * P])
                    g_ts = []
                    as2_all = sbm.tile([P, T], F32, tag="as2a")
                    e2_ps = ps_tr.tile([P, T], F32, tag="e2", padded_shape=[P, 512])
                    nc.tensor.matmul(e2_ps[:], lhsT=zl_s[:], rhs=zr_s[:, 0:T],
                                     start=True, stop=False)
                    for j in range(T):
                        g_t = sbg.tile([P, GW], F32, tag="gt")
                        nc.gpsimd.indirect_dma_start(
                            out=g_t[:], out_offset=None, in_=g_all[:, :],
                            in_offset=IndirectOffsetOnAxis(
                                ap=srcg[:, b * T + j:b * T + j + 1], axis=0))
                        g_ts.append(g_t)
                        nc.vector.tensor_copy(as2_all[:, j:j + 1], g_t[:, 129:130])
                        nc.tensor.matmul(e2_ps[:, j:j + 1],
                                         lhsT=mt_b[:, j * P:(j + 1) * P],
                                         rhs=ad2_h[:, b:b + 1], start=False,
                                         stop=(j == T - 1))
                    s2_s = sbm.tile([P, T], F32, tag="s2s")
                    nc.vector.tensor_tensor(out=s2_s[:], in0=e2_ps[:], in1=as2_all[:],
                                            op=OP.add)
                    t2_s = sbm.tile([P, T], F32, tag="t2s")
                    nc.vector.tensor_scalar(out=t2_s[:], in0=s2_s[:], scalar1=SLOPE,
                                            scalar2=None, op0=OP.mult)
                    e2m = sbm.tile([P, T], F32, tag="e2m")
                    nc.vector.tensor_tensor(out=e2m[:], in0=s2_s[:], in1=t2_s[:],
                                            op=OP.max)
                    al2 = sbm.tile([P, T], F32, tag="al2")
                    nc.scalar.activation(al2[:], e2m[:], AF.Exp)

                    agg2_ps = ps_acc.tile([P, 129], F32, tag="agg2", padded_shape=[P, 512])
                    nc.tensor.matmul(agg2_ps[:], lhsT=zl_s[:], rhs=zr_s[:, 0:129],
                                     start=True, stop=False)
                    for j in range(T):
                        m_s = sbm.tile([P, P], F16, tag="m2")
                        nc.vector.tensor_tensor(
                            out=m_s[:],
                            in0=dstloc[:, b * T + j:b * T + j + 1].to_broadcast([P, P]),
                            in1=iota_f[:], op=OP.is_equal)
                        rhs2 = sbm.tile([P, 129], F16, tag="rhs2")
                        nc.vector.tensor_scalar(out=rhs2[:], in0=g_ts[j][:, 0:129],
                                                scalar1=al2[:, j:j + 1], scalar2=None,
                                                op0=OP.mult)
                        nc.tensor.matmul(agg2_ps[:], lhsT=m_s[:], rhs=rhs2[:],
                                         start=False, stop=(j == T - 1))
                    den2r = sbm.tile([P, 1], F32, tag="den2r")
                    nc.vector.tensor_scalar(out=den2r[:], in0=agg2_ps[:, 128:129],
                                            scalar1=1e-16, scalar2=None, op0=OP.add)
                    nc.vector.reciprocal(den2r[:], den2r[:])
                    y_s = sb.tile([P, EMB], F32, tag="y")
                    nc.vector.tensor_scalar(out=y_s[:], in0=agg2_ps[:, 0:128],
                                            scalar1=den2r[:, 0:1], scalar2=None,
                                            op0=OP.mult)
                    nc.vector.tensor_tensor(out=y_s[:], in0=y_s[:], in1=b2_bc[:],
                                            op=OP.add)
                    tt = sbm.tile([P, EMB], F32, tag="z_t")
                    rr = sbm.tile([P, EMB], F32, tag="z_r")
                    nc.scalar.activation(tt[:], y_s[:], AF.Exp)
                    nc.scalar.activation(rr[:], y_s[:], AF.Relu)
                    nc.vector.tensor_scalar(out=tt[:], in0=tt[:], scalar1=-1.0,
                                            scalar2=None, op0=OP.add)
                    z_sb = sb.tile([P, EMB], F32, tag="zsb")
                    nc.vector.tensor_tensor(out=z_sb[:], in0=tt[:], in1=rr[:], op=OP.min)
                    nc.sync.dma_start(z_own[b * P:(b + 1) * P, :], z_sb[:])
                    nc.sync.dma_start(z_ext[b * P:(b + 1) * P, :], z_sb[:])
                    zT_ps = ps_tr.tile([P, P], F32, tag="ztail", padded_shape=[P, 512])
                    nc.tensor.transpose(zT_ps[:], z_sb[:], ident[:])
                    zT_s = sbm.tile([P, P], F32, tag="zTs")
                    nc.vector.tensor_copy(zT_s[:], zT_ps[:])
                    lg_ps = ps_tr.tile([P, 4], F32, tag="ztail", padded_shape=[P, 512])
                    nc.tensor.matmul(lg_ps[:], lhsT=zT_s[:], rhs=clsWT[:],
                                     start=True, stop=True)
                    nc.vector.tensor_tensor(out=lg_acc[:, b * 4:(b + 1) * 4],
                                            in0=lg_ps[:], in1=clsb_bc[:], op=OP.add)

            # ---------------- AllGather z, preds ----------------
            nc.gpsimd.collective_compute(
                "AllGather", mybir.AluOpType.bypass,
                replica_groups=[list(range(NC))],
                ins=[z_own.ap().opt()], outs=[z_all.ap().opt()])

            with tc.tile_pool(name="sb3", bufs=4) as sb:
                sums = cp.tile([P, npt], F32)
                for q in range(npt):
                    za = sb.tile([P, EMB], F32, tag="za")
                    nc.gpsimd.indirect_dma_start(
                        out=za[:], out_offset=None, in_=z_all[:, :],
                        in_offset=IndirectOffsetOnAxis(ap=pa_t[:, q:q + 1], axis=0))
                    zb = sb.tile([P, EMB], F32, tag="zb")
                    nc.gpsimd.indirect_dma_start(
                        out=zb[:], out_offset=None, in_=z_all[:, :],
                        in_offset=IndirectOffsetOnAxis(ap=pb_t[:, q:q + 1], axis=0))
                    mm = sb.tile([P, EMB], F32, tag="zm")
                    nc.vector.tensor_tensor(out=mm[:], in0=za[:], in1=zb[:],
                                            op=OP.mult)
                    nc.vector.reduce_sum(out=sums[:, q:q + 1], in_=mm[:],
                                         axis=mybir.AxisListType.X)
                nc.scalar.activation(pr_acc[:], sums[:], AF.Sigmoid)
            nc.sync.dma_start(lg_ext[:, :], lg_acc[:])
            nc.sync.dma_start(pr_ext[:, :], pr_acc[:])
    nc.compile()
    return nc


_LAST_EXEC_NS = None


def _trace_kwargs():
    import os
    if not os.environ.get("GAT_TRACE"):
        return {}
    import sys, types
    try:
        import antenv.axon_hooks  # noqa: F401
    except ImportError:
        import antenv
        mod = types.ModuleType("antenv.axon_hooks")
        mod._hook = None
        mod.set_axon_ntff_profile_hook = lambda h: setattr(mod, "_hook", h)
        mod.get_axon_ntff_profile_hook = lambda: mod._hook
        sys.modules["antenv.axon_hooks"] = mod
        antenv.axon_hooks = mod
        try:
            from trn_agent_boot.trn_boot import _ntff_profile_via_ctypes
            mod.set_axon_ntff_profile_hook(
                _ntff_profile_via_ctypes("/opt/axon/libaxon_pjrt.so"))
        except Exception:
            return {}
    return {"trace": True}


def kernel(**inputs):
    global _LAST_EXEC_NS
    from concourse.bass_utils import run_bass_kernel_spmd
    per_core, t_fix, npt, pp, rowv = _prep(**inputs)
    nc = _build(t_fix, npt)
    res = run_bass_kernel_spmd(nc, per_core, core_ids=list(range(NC)),
                               **_trace_kwargs())
    _LAST_EXEC_NS = res.exec_time_ns
    z_cat = np.concatenate([res.results[c]["z_out"] for c in range(NC)], axis=0)
    lg_cat = np.concatenate(
        [res.results[c]["lg_out"].reshape(P, NB, 4).transpose(1, 0, 2)
         .reshape(BLKP, 4) for c in range(NC)], axis=0)
    z = z_cat[rowv]
    logits = lg_cat[rowv]
    preds = np.concatenate(
        [res.results[c]["pr_out"].T.reshape(npt * P)[:pp] for c in range(NC)])
    return z.astype(np.float32), logits.astype(np.float32), preds.astype(np.float32)


# revision 18
# speedup vs baseline: 1.2585x; 1.1614x over previous
"""2-layer GAT (GATConv x2 + link predictor) on 8 Trainium2 NeuronCores.

Sharding: nodes partitioned into 8 contiguous blocks (graph parallel).
Each core aggregates incoming edges of its own destination block.
L1 source features are host-pre-gathered into edge-slot order (x is an
input); L2 aggregates the device-computed g-table (AllGather'd across
cores) with device-side indirect-DMA gathers, and link predictions
gather rows of the AllGather'd z-table.

Segment softmax / segment sum are done with per-tile mask matmuls on the
tensor engine: edges are sorted by destination and padded per 128-node
destination tile; M[e,d] = (dstloc[e]==d) maps 128 edges onto the tile's
nodes, so M.T @ (alpha * feat) accumulates per-node sums in PSUM.
"""
import numpy as np

# model dims (from the reference problem; fixed by the harness)
N_NODES = 50000
DIM = 128
HEADS = 4
HID = 256
EMB = 128
SLOPE = 0.2

NC = 8
P = 128
NB = 50                        # dst tiles (node groups) per core
GRP = N_NODES // (NC * NB)     # 125 real nodes per group (3 pad slots)
BLKP = NB * P                  # 6400 padded rows per core
GW = 131                       # g-table row: [g(128) | 1.0 | as2 | ad2]
PAD_DST = 999.0


def _prep(x, e, p, n, W1, a_src1, a_dst1, b1, W2, a_src2, a_dst2, b2, cls_W, cls_b):
    """Host-side graph/index/layout prep. No model FLOPs on node data."""
    x = np.asarray(x, np.float32)
    e = np.asarray(e, np.int64)
    p = np.asarray(p, np.int64)
    n = np.asarray(n, np.int64)

    loop = np.arange(N_NODES, dtype=np.int64)
    src = np.concatenate([e[0], loop])
    dst = np.concatenate([e[1], loop])

    # degree-balanced node grouping: snake-assign nodes (sorted by in-degree)
    # to NC*NB groups of GRP nodes so per-group edge counts are uniform.
    deg = np.bincount(dst, minlength=N_NODES)
    by_deg = np.argsort(-deg, kind="stable")
    ngrp = NC * NB
    grp = np.zeros(N_NODES, np.int64)
    pos = np.zeros(N_NODES, np.int64)
    for pss in range(GRP):
        seg = by_deg[pss * ngrp:(pss + 1) * ngrp]
        gids = np.arange(ngrp) if pss % 2 == 0 else np.arange(ngrp)[::-1]
        grp[seg] = gids
        pos[seg] = pss
    rowv = grp * P + pos                      # node -> padded table row
    row_of = lambda g: rowv[g]

    # per (core, dst-tile) edge lists
    key = grp[dst]
    core_of = key // NB
    order = np.lexsort((dst, key))
    src_s, dst_s = src[order], dst[order]
    counts = np.bincount(key, minlength=NC * NB)
    t_fix = int(np.max((counts + P - 1) // P))
    nT = NB * t_fix
    n_slots = nT * P

    starts = np.zeros(NC * NB + 1, dtype=np.int64)
    np.cumsum(counts, out=starts[1:])

    xe = np.zeros((NC, n_slots, DIM), np.float32)
    dstloc_sl = np.full((NC, n_slots), PAD_DST, np.float32)
    srcg_sl = np.zeros((NC, n_slots), np.int32)
    for c in range(NC):
        for b in range(NB):
            k = c * NB + b
            cnt = counts[k]
            if cnt == 0:
                continue
            s0 = starts[k]
            base = b * t_fix * P
            sl = slice(base, base + cnt)
            es = src_s[s0:s0 + cnt]
            xe[c, sl] = x[es]
            dstloc_sl[c, sl] = pos[dst_s[s0:s0 + cnt]]
            srcg_sl[c, sl] = rowv[es]
    # slot s lives at [partition s%128, tile s//128]
    dstloc = dstloc_sl.reshape(NC, nT, P).transpose(0, 2, 1).copy()   # [NC,P,nT]
    srcg = srcg_sl.reshape(NC, nT, P).transpose(0, 2, 1).copy()
    xeT = np.ascontiguousarray(xe.transpose(0, 2, 1).astype(np.float16))  # [NC,DIM,n_slots]
    # transposed masks MT[d, e] per tile, laid out [P, nT*P]
    dl = dstloc_sl.reshape(NC, nT, P)                                 # [c, t, e]
    mtb = (dl[:, :, None, :] == np.arange(P, dtype=np.float32)[None, None, :, None])
    mtb = np.ascontiguousarray(
        mtb.astype(np.float16).transpose(0, 2, 1, 3).reshape(NC, P, nT * P))

    # own-block x transposed (relabeled rows), padded
    xall = np.zeros((NC * BLKP, DIM), np.float32)
    xall[rowv] = x
    xTo = np.zeros((NC, DIM, BLKP), np.float32)
    for c in range(NC):
        xTo[c] = xall[c * BLKP:(c + 1) * BLKP].T

    # pred edges sharded by position
    n_pred = p.shape[1] + n.shape[1]
    pp = n_pred // NC
    npt = (pp + P - 1) // P
    pa = np.zeros((NC, npt * P), np.int32)
    pb = np.zeros((NC, npt * P), np.int32)
    allp = np.concatenate([p, n], axis=1)
    for c in range(NC):
        seg = allp[:, c * pp:(c + 1) * pp]
        pa[c, :pp] = row_of(seg[0])
        pb[c, :pp] = row_of(seg[1])
    pa = pa.reshape(NC, npt, P).transpose(0, 2, 1).copy()
    pb = pb.reshape(NC, npt, P).transpose(0, 2, 1).copy()

    # weight prep (weights only)
    W1 = np.asarray(W1, np.float32)
    W2 = np.asarray(W2, np.float32)
    W1h = W1.reshape(HEADS, HID, DIM)
    was1 = np.einsum('kh,khd->kd', np.asarray(a_src1, np.float32), W1h)
    wad1 = np.einsum('kh,khd->kd', np.asarray(a_dst1, np.float32), W1h)
    W1T = np.ascontiguousarray(W1.T)                                   # [128, 1024]
    b1 = np.asarray(b1, np.float32)
    b1T = b1.reshape(8, P).T.copy()                                    # [128, 8] col c
    W2T = np.ascontiguousarray(W2.T)                                   # [1024, 128]
    was2 = W2T @ np.asarray(a_src2, np.float32)[0]                     # [1024]
    wad2 = W2T @ np.asarray(a_dst2, np.float32)[0]
    W2aug = np.zeros((8, P, 130), np.float32)
    for c in range(8):
        W2aug[c, :, :128] = W2T[c * P:(c + 1) * P]
        W2aug[c, :, 128] = was2[c * P:(c + 1) * P]
        W2aug[c, :, 129] = wad2[c * P:(c + 1) * P]
    W2augP = np.ascontiguousarray(W2aug.transpose(1, 0, 2)).reshape(P, 8 * 130)
    b2_bc = np.tile(np.asarray(b2, np.float32)[None, :], (P, 1))       # [128, 128]
    clsWT = np.ascontiguousarray(np.asarray(cls_W, np.float32).T)      # [128, 4]
    clsb_bc = np.tile(np.asarray(cls_b, np.float32)[None, :], (P, 1))  # [128, 4]
    iota_f = np.tile(np.arange(P, dtype=np.float32)[None, :], (P, 1))
    ident = np.eye(P, dtype=np.float32)

    shared = dict(W1T=W1T.astype(np.float16), wasT1=np.ascontiguousarray(was1.T.astype(np.float16)),
                  wadT1=np.ascontiguousarray(wad1.T), b1T=b1T,
                  W2augP=W2augP.astype(np.float16), b2_bc=b2_bc, clsWT=clsWT, clsb_bc=clsb_bc,
                  iota_f=iota_f, ident=ident, ident16=ident.astype(np.float16))
    per_core = []
    for c in range(NC):
        m = dict(shared)
        m.update(xe=xe[c], xeT=xeT[c], xTo=xTo[c], dstloc=dstloc[c],
                 srcg=srcg[c], pa=pa[c], pb=pb[c], mtb=mtb[c])
        per_core.append(m)
    return per_core, t_fix, npt, pp, rowv


def _build(t_fix, npt):
    import concourse.bacc as bacc
    import concourse.mybir as mybir
    import concourse.tile as tile
    from concourse.bass import IndirectOffsetOnAxis

    F32 = mybir.dt.float32
    F16 = mybir.dt.float16
    AF = mybir.ActivationFunctionType
    OP = mybir.AluOpType
    T = t_fix
    nT = NB * T
    n_slots = nT * P

    nc = bacc.Bacc("TRN2", target_bir_lowering=False, debug=False, num_devices=NC)
    din = {}
    for name, shape, dt in [
        ("xe", [n_slots, DIM], F32), ("xeT", [DIM, n_slots], F16),
        ("mtb", [P, n_slots], F16),
        ("xTo", [DIM, BLKP], F32), ("dstloc", [P, nT], F32),
        ("srcg", [P, nT], mybir.dt.int32),
        ("pa", [P, npt], mybir.dt.int32), ("pb", [P, npt], mybir.dt.int32),
        ("W1T", [P, HEADS * HID], F16), ("wasT1", [P, HEADS], F16),
        ("wadT1", [P, HEADS], F32), ("b1T", [P, 8], F32),
        ("W2augP", [P, 8 * 130], F16), ("b2_bc", [P, EMB], F32),
        ("clsWT", [P, 4], F32), ("clsb_bc", [P, 4], F32),
        ("iota_f", [P, P], F32), ("ident", [P, P], F32), ("ident16", [P, P], F16),
    ]:
        din[name] = nc.dram_tensor(name, shape, dt, kind="ExternalInput")
    z_ext = nc.dram_tensor("z_out", [BLKP, EMB], F32, kind="ExternalOutput")
    lg_ext = nc.dram_tensor("lg_out", [P, NB * 4], F32, kind="ExternalOutput")
    pr_ext = nc.dram_tensor("pr_out", [P, npt], F32, kind="ExternalOutput")

    g_own = nc.dram_tensor("g_own", [BLKP, GW], F32)
    g_all = nc.dram_tensor("g_all", [NC * BLKP, GW], F32, addr_space="Shared")
    z_own = nc.dram_tensor("z_own", [BLKP, EMB], F32)
    z_all = nc.dram_tensor("z_all", [NC * BLKP, EMB], F32, addr_space="Shared")

    with tile.TileContext(nc) as tc:
        with tc.tile_pool(name="const", bufs=1) as cp:
            W1T = cp.tile_from(din["W1T"].ap())
            wasT1 = cp.tile_from(din["wasT1"].ap())
            wadT1 = cp.tile_from(din["wadT1"].ap())
            b1T = cp.tile_from(din["b1T"].ap())
            W2augP = cp.tile_from(din["W2augP"].ap())
            b2_bc = cp.tile_from(din["b2_bc"].ap())
            clsWT = cp.tile_from(din["clsWT"].ap())
            clsb_bc = cp.tile_from(din["clsb_bc"].ap())
            iota_f = cp.tile_from(din["iota_f"].ap())
            ident = cp.tile_from(din["ident"].ap())
            ident16 = cp.tile_from(din["ident16"].ap())
            dstloc = cp.tile_from(din["dstloc"].ap())
            srcg = cp.tile_from(din["srcg"].ap())
            pa_t = cp.tile_from(din["pa"].ap())
            pb_t = cp.tile_from(din["pb"].ap())
            zl_s = cp.tile([1, P], F32)
            nc.vector.memset(zl_s[:], 0.0)
            zr_s = cp.tile([1, 512], F32)
            nc.vector.memset(zr_s[:], 0.0)
            ad2_sb = cp.tile([P, NB], F32)
            lg_acc = cp.tile([P, NB * 4], F32)
            pr_acc = cp.tile([P, npt], F32)

            # ---------------- L1 + g-table ----------------
            with tc.tile_pool(name="sb1", bufs=2) as sb, \
                 tc.tile_pool(name="sbm", bufs=3) as sbm, \
                 tc.tile_pool(name="ps_acc", bufs=2, space="PSUM") as ps_acc, \
                 tc.tile_pool(name="ps_tr", bufs=2, space="PSUM") as ps_tr, \
                 tc.tile_pool(name="ps_o", bufs=1, space="PSUM") as ps_o:
                for b in range(NB):
                    e0 = b * T * P
                    xe_b = sb.tile([P, T * DIM], F32, tag="xe")
                    nc.sync.dma_start(
                        xe_b[:].rearrange("p (t d) -> p t d", t=T),
                        din["xe"][e0:e0 + T * P, :].rearrange("(t p) d -> p t d", p=P))
                    xeT_b = sb.tile([P, T * P], F16, tag="xeT")
                    nc.sync.dma_start(xeT_b[:], din["xeT"][:, e0:e0 + T * P])
                    mt_b = sb.tile([P, T * P], F16, tag="mt")
                    nc.sync.dma_start(mt_b[:], din["mtb"][:, e0:e0 + T * P])
                    xTo_t = sbm.tile([P, P], F32, tag="xTo")
                    nc.sync.dma_start(xTo_t[:], din["xTo"][:, b * P:(b + 1) * P])

                    ad1_ps = ps_tr.tile([P, HEADS], F32, tag="asad", padded_shape=[P, 512])
                    nc.tensor.matmul(ad1_ps[:], lhsT=xTo_t[:], rhs=wadT1[:],
                                     start=True, stop=True)
                    ad1_s = sbm.tile([P, HEADS], F16, tag="ad1s")
                    nc.vector.tensor_copy(ad1_s[:], ad1_ps[:])

                    # batched as+ad for all T tiles: [e, 4*T]
                    asad_ps = ps_tr.tile([P, 4 * T], F32, tag="asad", padded_shape=[P, 512])
                    nc.tensor.matmul(asad_ps[:], lhsT=zl_s[:], rhs=zr_s[:, 0:4 * T],
                                     start=True, stop=False)
                    for j in range(T):
                        nc.tensor.matmul(asad_ps[:, 4 * j:4 * j + 4],
                                         lhsT=xeT_b[:, j * P:(j + 1) * P],
                                         rhs=wasT1[:], start=False, stop=False)
                        nc.tensor.matmul(asad_ps[:, 4 * j:4 * j + 4],
                                         lhsT=mt_b[:, j * P:(j + 1) * P],
                                         rhs=ad1_s[:], start=False,
                                         stop=(j == T - 1))
                    t02 = sbm.tile([P, 4 * T], F32, tag="t02")
                    nc.vector.tensor_scalar(out=t02[:], in0=asad_ps[:], scalar1=SLOPE,
                                            scalar2=None, op0=OP.mult)
                    e_s = sbm.tile([P, 4 * T], F32, tag="e")
                    nc.vector.tensor_tensor(out=e_s[:], in0=asad_ps[:], in1=t02[:],
                                            op=OP.max)
                    al_s = sbm.tile([P, 4 * T], F32, tag="al")
                    nc.scalar.activation(al_s[:], e_s[:], AF.Exp)

                    agg01 = ps_acc.tile([P, 258], F32, tag="agg01", padded_shape=[P, 512])
                    agg23 = ps_acc.tile([P, 258], F32, tag="agg23", padded_shape=[P, 512])
                    nc.tensor.matmul(agg01[:], lhsT=zl_s[:], rhs=zr_s[:, 0:258],
                                     start=True, stop=False)
                    nc.tensor.matmul(agg23[:], lhsT=zl_s[:], rhs=zr_s[:, 0:258],
                                     start=True, stop=False)
                    for j in range(T):
                        m_s = sbm.tile([P, P], F16, tag="m")
                        nc.vector.tensor_tensor(
                            out=m_s[:],
                            in0=dstloc[:, b * T + j:b * T + j + 1].to_broadcast([P, P]),
                            in1=iota_f[:], op=OP.is_equal)
                        xw01 = sbm.tile([P, 258], F16, tag="xw01")
                        xw23 = sbm.tile([P, 258], F16, tag="xw23")
                        for k in range(4):
                            dstt = (xw01, xw23)[k // 2]
                            nc.vector.tensor_scalar(
                                out=dstt[:, (k % 2) * P:(k % 2) * P + P],
                                in0=xe_b[:, j * DIM:(j + 1) * DIM],
                                scalar1=al_s[:, 4 * j + k:4 * j + k + 1],
                                scalar2=None, op0=OP.mult)
                        nc.vector.tensor_copy(xw01[:, 256:258],
                                              al_s[:, 4 * j:4 * j + 2])
                        nc.vector.tensor_copy(xw23[:, 256:258],
                                              al_s[:, 4 * j + 2:4 * j + 4])
                        last = (j == T - 1)
                        nc.tensor.matmul(agg01[:], lhsT=m_s[:], rhs=xw01[:],
                                         start=False, stop=last)
                        nc.tensor.matmul(agg23[:], lhsT=m_s[:], rhs=xw23[:],
                                         start=False, stop=last)

                    # block tail
                    denr = sbm.tile([P, HEADS], F32, tag="denr")
                    nc.vector.tensor_scalar(out=denr[:, 0:2], in0=agg01[:, 256:258],
                                            scalar1=1e-16, scalar2=None, op0=OP.add)
                    nc.vector.tensor_scalar(out=denr[:, 2:4], in0=agg23[:, 256:258],
                                            scalar1=1e-16, scalar2=None, op0=OP.add)
                    nc.vector.reciprocal(denr[:], denr[:])
                    aggn = sb.tile([P, 4 * P], F16, tag="aggn")
                    for k in range(4):
                        srct = (agg01, agg23)[k // 2]
                        nc.vector.tensor_scalar(
                            out=aggn[:, k * P:(k + 1) * P],
                            in0=srct[:, (k % 2) * P:(k % 2) * P + P],
                            scalar1=denr[:, k:k + 1], scalar2=None, op0=OP.mult)
                    aggT_ps = ps_o.tile([P, 4 * P], F16, tag="tail", padded_shape=[P, 1024])
                    for k in range(4):
                        nc.tensor.transpose(aggT_ps[:, k * P:(k + 1) * P],
                                            aggn[:, k * P:(k + 1) * P], ident16[:])
                    aggT_s = sb.tile([P, 4 * P], F16, tag="aggTs")
                    nc.vector.tensor_copy(aggT_s[:], aggT_ps[:])

                    o1_ps = ps_o.tile([P, 8 * P], F32, tag="tail", padded_shape=[P, 1024])
                    for c in range(8):
                        nc.tensor.matmul(
                            o1_ps[:, c * P:(c + 1) * P],
                            lhsT=W1T[:, c * P:(c + 1) * P],
                            rhs=aggT_s[:, (c // 2) * P:(c // 2 + 1) * P],
                            start=True, stop=True)
                    h1T = sb.tile([P, 8 * P], F16, tag="h1T")
                    for c in range(8):
                        tt = sbm.tile([P, P], F32, tag="elu_t")
                        rr = sbm.tile([P, P], F32, tag="elu_r")
                        nc.scalar.activation(tt[:], o1_ps[:, c * P:(c + 1) * P],
                                             AF.Exp, bias=b1T[:, c:c + 1])
                        nc.scalar.activation(rr[:], o1_ps[:, c * P:(c + 1) * P],
                                             AF.Relu, bias=b1T[:, c:c + 1])
                        nc.vector.tensor_scalar(out=tt[:], in0=tt[:], scalar1=-1.0,
                                                scalar2=None, op0=OP.add)
                        nc.vector.tensor_tensor(out=h1T[:, c * P:(c + 1) * P],
                                                in0=tt[:], in1=rr[:], op=OP.min)
                    gaug_ps = ps_o.tile([P, 130], F32, tag="tail", padded_shape=[P, 1024])
                    for c in range(8):
                        nc.tensor.matmul(gaug_ps[:], lhsT=h1T[:, c * P:(c + 1) * P],
                                         rhs=W2augP[:, c * 130:(c + 1) * 130],
                                         start=(c == 0), stop=(c == 7))
                    g_sb = sb.tile([P, GW], F32, tag="gsb")
                    nc.vector.tensor_copy(g_sb[:, 0:128], gaug_ps[:, 0:128])
                    nc.vector.memset(g_sb[:, 128:129], 1.0)
                    nc.vector.tensor_copy(g_sb[:, 129:131], gaug_ps[:, 128:130])
                    nc.vector.tensor_copy(ad2_sb[:, b:b + 1], gaug_ps[:, 129:130])
                    nc.sync.dma_start(g_own[b * P:(b + 1) * P, :], g_sb[:])

            # ---------------- AllGather g ----------------
            nc.gpsimd.collective_compute(
                "AllGather", mybir.AluOpType.bypass,
                replica_groups=[list(range(NC))],
                ins=[g_own.ap().opt()], outs=[g_all.ap().opt()])

            # ---------------- L2 ----------------
            ad2_h = cp.tile([P, NB], F16)
            nc.vector.tensor_copy(ad2_h[:], ad2_sb[:])
            with tc.tile_pool(name="sb2", bufs=2) as sb, \
                 tc.tile_pool(name="sbg", bufs=2 * T + 2) as sbg, \
                 tc.tile_pool(name="sbm2", bufs=3) as sbm, \
                 tc.tile_pool(name="ps2_acc", bufs=2, space="PSUM") as ps_acc, \
                 tc.tile_pool(name="ps2_tr", bufs=2, space="PSUM") as ps_tr:
                for b in range(NB):
                    mt_b = sb.tile([P, T * P], F16, tag="mt2")
                    e0 = b * T * P
                    nc.sync.dma_start(mt_b[:], din["mtb"][:, e0:e0 + T * P])
                    g_ts = []
                    as2_all = sbm.tile([P, T], F32, tag="as2a")
                    e2_ps = ps_tr.tile([P, T], F32, tag="e2", padded_shape=[P, 512])
                    nc.tensor.matmul(e2_ps[:], lhsT=zl_s[:], rhs=zr_s[:, 0:T],
                                     start=True, stop=False)
                    for j in range(T):
                        g_t = sbg.tile([P, GW], F32, tag="gt")
                        nc.gpsimd.indirect_dma_start(
                            out=g_t[:], out_offset=None, in_=g_all[:, :],
                            in_offset=IndirectOffsetOnAxis(
                                ap=srcg[:, b * T + j:b * T + j + 1], axis=0))
                        g_ts.append(g_t)
                        nc.vector.tensor_copy(as2_all[:, j:j + 1], g_t[:, 129:130])
                        nc.tensor.matmul(e2_ps[:, j:j + 1],
                                         lhsT=mt_b[:, j * P:(j + 1) * P],
                                         rhs=ad2_h[:, b:b + 1], start=False,
                                         stop=(j == T - 1))
                    s2_s = sbm.tile([P, T], F32, tag="s2s")
                    nc.vector.tensor_tensor(out=s2_s[:], in0=e2_ps[:], in1=as2_all[:],
                                            op=OP.add)
                    t2_s = sbm.tile([P, T], F32, tag="t2s")
                    nc.vector.tensor_scalar(out=t2_s[:], in0=s2_s[:], scalar1=SLOPE,
                                            scalar2=None, op0=OP.mult)
                    e2m = sbm.tile([P, T], F32, tag="e2m")
                    nc.vector.tensor_tensor(out=e2m[:], in0=s2_s[:], in1=t2_s[:],
                                            op=OP.max)
                    al2 = sbm.tile([P, T], F32, tag="al2")
                    nc.scalar.activation(al2[:], e2m[:], AF.Exp)

                    agg2_ps = ps_acc.tile([P, 129], F32, tag="agg2", padded_shape=[P, 512])
                    nc.tensor.matmul(agg2_ps[:], lhsT=zl_s[:], rhs=zr_s[:, 0:129],
                                     start=True, stop=False)
                    for j in range(T):
                        m_s = sbm.tile([P, P], F16, tag="m2")
                        nc.vector.tensor_tensor(
                            out=m_s[:],
                            in0=dstloc[:, b * T + j:b * T + j + 1].to_broadcast([P, P]),
                            in1=iota_f[:], op=OP.is_equal)
                        rhs2 = sbm.tile([P, 129], F16, tag="rhs2")
                        nc.vector.tensor_scalar(out=rhs2[:], in0=g_ts[j][:, 0:129],
                                                scalar1=al2[:, j:j + 1], scalar2=None,
                                                op0=OP.mult)
                        nc.tensor.matmul(agg2_ps[:], lhsT=m_s[:], rhs=rhs2[:],
                                         start=False, stop=(j == T - 1))
                    den2r = sbm.tile([P, 1], F32, tag="den2r")
                    nc.vector.tensor_scalar(out=den2r[:], in0=agg2_ps[:, 128:129],
                                            scalar1=1e-16, scalar2=None, op0=OP.add)
                    nc.vector.reciprocal(den2r[:], den2r[:])
                    y_s = sb.tile([P, EMB], F32, tag="y")
                    nc.vector.tensor_scalar(out=y_s[:], in0=agg2_ps[:, 0:128],
                                            scalar1=den2r[:, 0:1], scalar2=None,
                                            op0=OP.mult)
                    nc.vector.tensor_tensor(out=y_s[:], in0=y_s[:], in1=b2_bc[:],
                                            op=OP.add)
                    tt = sbm.tile([P, EMB], F32, tag="z_t")
                    rr = sbm.tile([P, EMB], F32, tag="z_r")
                    nc.scalar.activation(tt[:], y_s[:], AF.Exp)
                    nc.scalar.activation(rr[:], y_s[:], AF.Relu)
                    nc.vector.tensor_scalar(out=tt[:], in0=tt[:], scalar1=-1.0,
                                            scalar2=None, op0=OP.add)
                    z_sb = sb.tile([P, EMB], F32, tag="zsb")
                    nc.vector.tensor_tensor(out=z_sb[:], in0=tt[:], in1=rr[:], op=OP.min)
                    nc.sync.dma_start(z_own[b * P:(b + 1) * P, :], z_sb[:])
                    nc.sync.dma_start(z_ext[b * P:(b + 1) * P, :], z_sb[:])
                    zT_ps = ps_tr.tile([P, P], F32, tag="ztail", padded_shape=[P, 512])
                    nc.tensor.transpose(zT_ps[:], z_sb[:], ident[:])
                    zT_s = sbm.tile([P, P], F32, tag="zTs")
                    nc.vector.tensor_copy(zT_s[:], zT_ps[:])
                    lg_ps = ps_tr.tile([P, 4], F32, tag="ztail", padded_shape=[P, 512])
                    nc.tensor.matmul(lg_ps[:], lhsT=zT_s[:], rhs=clsWT[:],
                                     start=True, stop=True)
                    nc.vector.tensor_tensor(out=lg_acc[:, b * 4:(b + 1) * 4],
                                            in0=lg_ps[:], in1=clsb_bc[:], op=OP.add)

            # ---------------- AllGather z, preds ----------------
            nc.gpsimd.collective_compute(
                "AllGather", mybir.AluOpType.bypass,
                replica_groups=[list(range(NC))],
                ins=[z_own.ap().opt()], outs=[z_all.ap().opt()])

            with tc.tile_pool(name="sb3", bufs=4) as sb:
                sums = cp.tile([P, npt], F32)
                for q in range(npt):
                    za = sb.tile([P, EMB], F32, tag="za")
                    nc.gpsimd.indirect_dma_start(
                        out=za[:], out_offset=None, in_=z_all[:, :],
                        in_offset=IndirectOffsetOnAxis(ap=pa_t[:, q:q + 1], axis=0))
                    zb = sb.tile([P, EMB], F32, tag="zb")
                    nc.gpsimd.indirect_dma_start(
                        out=zb[:], out_offset=None, in_=z_all[:, :],
                        in_offset=IndirectOffsetOnAxis(ap=pb_t[:, q:q + 1], axis=0))
                    mm = sb.tile([P, EMB], F32, tag="zm")
                    nc.vector.tensor_tensor(out=mm[:], in0=za[:], in1=zb[:],
                                            op=OP.mult)
                    nc.vector.reduce_sum(out=sums[:, q:q + 1], in_=mm[:],
                                         axis=mybir.AxisListType.X)
                nc.scalar.activation(pr_acc[:], sums[:], AF.Sigmoid)
            nc.sync.dma_start(lg_ext[:, :], lg_acc[:])
            nc.sync.dma_start(pr_ext[:, :], pr_acc[:])
    nc.compile()
    return nc


_LAST_EXEC_NS = None


def _trace_kwargs():
    import os
    if not os.environ.get("GAT_TRACE"):
        return {}
    import sys, types
    try:
        import antenv.axon_hooks  # noqa: F401
    except ImportError:
        import antenv
        mod = types.ModuleType("antenv.axon_hooks")
        mod._hook = None
        mod.set_axon_ntff_profile_hook = lambda h: setattr(mod, "_hook", h)
        mod.get_axon_ntff_profile_hook = lambda: mod._hook
        sys.modules["antenv.axon_hooks"] = mod
        antenv.axon_hooks = mod
        try:
            from trn_agent_boot.trn_boot import _ntff_profile_via_ctypes
            mod.set_axon_ntff_profile_hook(
                _ntff_profile_via_ctypes("/opt/axon/libaxon_pjrt.so"))
        except Exception:
            return {}
    return {"trace": True}


def kernel(**inputs):
    global _LAST_EXEC_NS
    from concourse.bass_utils import run_bass_kernel_spmd
    per_core, t_fix, npt, pp, rowv = _prep(**inputs)
    nc = _build(t_fix, npt)
    res = run_bass_kernel_spmd(nc, per_core, core_ids=list(range(NC)),
                               **_trace_kwargs())
    _LAST_EXEC_NS = res.exec_time_ns
    z_cat = np.concatenate([res.results[c]["z_out"] for c in range(NC)], axis=0)
    lg_cat = np.concatenate(
        [res.results[c]["lg_out"].reshape(P, NB, 4).transpose(1, 0, 2)
         .reshape(BLKP, 4) for c in range(NC)], axis=0)
    z = z_cat[rowv]
    logits = lg_cat[rowv]
    preds = np.concatenate(
        [res.results[c]["pr_out"].T.reshape(npt * P)[:pp] for c in range(NC)])
    return z.astype(np.float32), logits.astype(np.float32), preds.astype(np.float32)


# revision 19
# speedup vs baseline: 1.3615x; 1.0819x over previous
"""2-layer GAT (GATConv x2 + link predictor) on 8 Trainium2 NeuronCores.

Sharding: nodes partitioned into 8 contiguous blocks (graph parallel).
Each core aggregates incoming edges of its own destination block.
L1 source features are host-pre-gathered into edge-slot order (x is an
input); L2 aggregates the device-computed g-table (AllGather'd across
cores) with device-side indirect-DMA gathers, and link predictions
gather rows of the AllGather'd z-table.

Segment softmax / segment sum are done with per-tile mask matmuls on the
tensor engine: edges are sorted by destination and padded per 128-node
destination tile; M[e,d] = (dstloc[e]==d) maps 128 edges onto the tile's
nodes, so M.T @ (alpha * feat) accumulates per-node sums in PSUM.
"""
import numpy as np

# model dims (from the reference problem; fixed by the harness)
N_NODES = 50000
DIM = 128
HEADS = 4
HID = 256
EMB = 128
SLOPE = 0.2

NC = 8
P = 128
NB = 50                        # dst tiles (node groups) per core
GRP = N_NODES // (NC * NB)     # 125 real nodes per group (3 pad slots)
BLKP = NB * P                  # 6400 padded rows per core
GW = 131                       # g-table row: [g(128) | 1.0 | as2 | ad2]
PAD_DST = 999.0


def _prep(x, e, p, n, W1, a_src1, a_dst1, b1, W2, a_src2, a_dst2, b2, cls_W, cls_b):
    """Host-side graph/index/layout prep. No model FLOPs on node data."""
    x = np.asarray(x, np.float32)
    e = np.asarray(e, np.int64)
    p = np.asarray(p, np.int64)
    n = np.asarray(n, np.int64)

    loop = np.arange(N_NODES, dtype=np.int64)
    src = np.concatenate([e[0], loop])
    dst = np.concatenate([e[1], loop])

    # degree-balanced node grouping: snake-assign nodes (sorted by in-degree)
    # to NC*NB groups of GRP nodes so per-group edge counts are uniform.
    deg = np.bincount(dst, minlength=N_NODES)
    by_deg = np.argsort(-deg, kind="stable")
    ngrp = NC * NB
    grp = np.zeros(N_NODES, np.int64)
    pos = np.zeros(N_NODES, np.int64)
    for pss in range(GRP):
        seg = by_deg[pss * ngrp:(pss + 1) * ngrp]
        gids = np.arange(ngrp) if pss % 2 == 0 else np.arange(ngrp)[::-1]
        grp[seg] = gids
        pos[seg] = pss
    rowv = grp * P + pos                      # node -> padded table row
    row_of = lambda g: rowv[g]

    # per (core, dst-tile) edge lists
    key = grp[dst]
    core_of = key // NB
    order = np.lexsort((dst, key))
    src_s, dst_s = src[order], dst[order]
    counts = np.bincount(key, minlength=NC * NB)
    t_fix = int(np.max((counts + P - 1) // P))
    nT = NB * t_fix
    n_slots = nT * P

    starts = np.zeros(NC * NB + 1, dtype=np.int64)
    np.cumsum(counts, out=starts[1:])

    xe = np.zeros((NC, n_slots, DIM), np.float32)
    dstloc_sl = np.full((NC, n_slots), PAD_DST, np.float32)
    srcg_sl = np.zeros((NC, n_slots), np.int32)
    for c in range(NC):
        for b in range(NB):
            k = c * NB + b
            cnt = counts[k]
            if cnt == 0:
                continue
            s0 = starts[k]
            base = b * t_fix * P
            sl = slice(base, base + cnt)
            es = src_s[s0:s0 + cnt]
            xe[c, sl] = x[es]
            dstloc_sl[c, sl] = pos[dst_s[s0:s0 + cnt]]
            srcg_sl[c, sl] = rowv[es]
    # slot s lives at [partition s%128, tile s//128]
    dstloc = dstloc_sl.reshape(NC, nT, P).transpose(0, 2, 1).copy()   # [NC,P,nT]
    srcg = srcg_sl.reshape(NC, nT, P).transpose(0, 2, 1).copy()
    xeT = np.ascontiguousarray(xe.transpose(0, 2, 1).astype(np.float16))  # [NC,DIM,n_slots]
    # transposed masks MT[d, e] per tile, laid out [P, nT*P]
    dl = dstloc_sl.reshape(NC, nT, P)                                 # [c, t, e]
    mtb = (dl[:, :, None, :] == np.arange(P, dtype=np.float32)[None, None, :, None])
    mtb = np.ascontiguousarray(
        mtb.astype(np.float16).transpose(0, 2, 1, 3).reshape(NC, P, nT * P))

    # own-block x transposed (relabeled rows), padded
    xall = np.zeros((NC * BLKP, DIM), np.float32)
    xall[rowv] = x
    xTo = np.zeros((NC, DIM, BLKP), np.float32)
    for c in range(NC):
        xTo[c] = xall[c * BLKP:(c + 1) * BLKP].T

    # pred edges sharded by position
    n_pred = p.shape[1] + n.shape[1]
    pp = n_pred // NC
    npt = (pp + P - 1) // P
    pa = np.zeros((NC, npt * P), np.int32)
    pb = np.zeros((NC, npt * P), np.int32)
    allp = np.concatenate([p, n], axis=1)
    for c in range(NC):
        seg = allp[:, c * pp:(c + 1) * pp]
        pa[c, :pp] = row_of(seg[0])
        pb[c, :pp] = row_of(seg[1])
    pa = pa.reshape(NC, npt, P).transpose(0, 2, 1).copy()
    pb = pb.reshape(NC, npt, P).transpose(0, 2, 1).copy()

    # weight prep (weights only)
    W1 = np.asarray(W1, np.float32)
    W2 = np.asarray(W2, np.float32)
    W1h = W1.reshape(HEADS, HID, DIM)
    was1 = np.einsum('kh,khd->kd', np.asarray(a_src1, np.float32), W1h)
    wad1 = np.einsum('kh,khd->kd', np.asarray(a_dst1, np.float32), W1h)
    W1T = np.ascontiguousarray(W1.T)                                   # [128, 1024]
    b1 = np.asarray(b1, np.float32)
    b1T = b1.reshape(8, P).T.copy()                                    # [128, 8] col c
    W2T = np.ascontiguousarray(W2.T)                                   # [1024, 128]
    was2 = W2T @ np.asarray(a_src2, np.float32)[0]                     # [1024]
    wad2 = W2T @ np.asarray(a_dst2, np.float32)[0]
    W2aug = np.zeros((8, P, 130), np.float32)
    for c in range(8):
        W2aug[c, :, :128] = W2T[c * P:(c + 1) * P]
        W2aug[c, :, 128] = was2[c * P:(c + 1) * P]
        W2aug[c, :, 129] = wad2[c * P:(c + 1) * P]
    W2augP = np.ascontiguousarray(W2aug.transpose(1, 0, 2)).reshape(P, 8 * 130)
    b2_bc = np.tile(np.asarray(b2, np.float32)[None, :], (P, 1))       # [128, 128]
    clsWT = np.ascontiguousarray(np.asarray(cls_W, np.float32).T)      # [128, 4]
    clsb_bc = np.tile(np.asarray(cls_b, np.float32)[None, :], (P, 1))  # [128, 4]
    iota_f = np.tile(np.arange(P, dtype=np.float32)[None, :], (P, 1))
    ident = np.eye(P, dtype=np.float32)

    shared = dict(W1T=W1T.astype(np.float16), wasT1=np.ascontiguousarray(was1.T.astype(np.float16)),
                  wadT1=np.ascontiguousarray(wad1.T), b1T=b1T,
                  W2augP=W2augP.astype(np.float16), b2_bc=b2_bc, clsWT=clsWT, clsb_bc=clsb_bc,
                  iota_f=iota_f, ident=ident, ident16=ident.astype(np.float16))
    per_core = []
    for c in range(NC):
        m = dict(shared)
        m.update(xe=xe[c].astype(np.float16), xeT=xeT[c], xTo=xTo[c], dstloc=dstloc[c],
                 srcg=srcg[c], pa=pa[c], pb=pb[c], mtb=mtb[c])
        per_core.append(m)
    return per_core, t_fix, npt, pp, rowv


def _build(t_fix, npt):
    import concourse.bacc as bacc
    import concourse.mybir as mybir
    import concourse.tile as tile
    from concourse.bass import IndirectOffsetOnAxis

    F32 = mybir.dt.float32
    F16 = mybir.dt.float16
    AF = mybir.ActivationFunctionType
    OP = mybir.AluOpType
    T = t_fix
    nT = NB * T
    n_slots = nT * P

    nc = bacc.Bacc("TRN2", target_bir_lowering=False, debug=False, num_devices=NC)
    din = {}
    for name, shape, dt in [
        ("xe", [n_slots, DIM], F16), ("xeT", [DIM, n_slots], F16),
        ("mtb", [P, n_slots], F16),
        ("xTo", [DIM, BLKP], F32), ("dstloc", [P, nT], F32),
        ("srcg", [P, nT], mybir.dt.int32),
        ("pa", [P, npt], mybir.dt.int32), ("pb", [P, npt], mybir.dt.int32),
        ("W1T", [P, HEADS * HID], F16), ("wasT1", [P, HEADS], F16),
        ("wadT1", [P, HEADS], F32), ("b1T", [P, 8], F32),
        ("W2augP", [P, 8 * 130], F16), ("b2_bc", [P, EMB], F32),
        ("clsWT", [P, 4], F32), ("clsb_bc", [P, 4], F32),
        ("iota_f", [P, P], F32), ("ident", [P, P], F32), ("ident16", [P, P], F16),
    ]:
        din[name] = nc.dram_tensor(name, shape, dt, kind="ExternalInput")
    z_ext = nc.dram_tensor("z_out", [BLKP, EMB], F32, kind="ExternalOutput")
    lg_ext = nc.dram_tensor("lg_out", [P, NB * 4], F32, kind="ExternalOutput")
    pr_ext = nc.dram_tensor("pr_out", [P, npt], F32, kind="ExternalOutput")

    g_own = nc.dram_tensor("g_own", [BLKP, GW], F16)
    g_all = nc.dram_tensor("g_all", [NC * BLKP, GW], F16, addr_space="Shared")
    z_own = nc.dram_tensor("z_own", [BLKP, EMB], F16)
    z_all = nc.dram_tensor("z_all", [NC * BLKP, EMB], F16, addr_space="Shared")

    with tile.TileContext(nc) as tc:
        with tc.tile_pool(name="const", bufs=1) as cp:
            W1T = cp.tile_from(din["W1T"].ap())
            wasT1 = cp.tile_from(din["wasT1"].ap())
            wadT1 = cp.tile_from(din["wadT1"].ap())
            b1T = cp.tile_from(din["b1T"].ap())
            W2augP = cp.tile_from(din["W2augP"].ap())
            b2_bc = cp.tile_from(din["b2_bc"].ap())
            clsWT = cp.tile_from(din["clsWT"].ap())
            clsb_bc = cp.tile_from(din["clsb_bc"].ap())
            iota_f = cp.tile_from(din["iota_f"].ap())
            ident = cp.tile_from(din["ident"].ap())
            ident16 = cp.tile_from(din["ident16"].ap())
            dstloc = cp.tile_from(din["dstloc"].ap())
            srcg = cp.tile_from(din["srcg"].ap())
            pa_t = cp.tile_from(din["pa"].ap())
            pb_t = cp.tile_from(din["pb"].ap())
            zl_s = cp.tile([1, P], F32)
            nc.vector.memset(zl_s[:], 0.0)
            zr_s = cp.tile([1, 512], F32)
            nc.vector.memset(zr_s[:], 0.0)
            ad2_sb = cp.tile([P, NB], F32)
            lg_acc = cp.tile([P, NB * 4], F32)
            pr_acc = cp.tile([P, npt], F32)

            # ---------------- L1 + g-table ----------------
            with tc.tile_pool(name="sb1", bufs=2) as sb, \
                 tc.tile_pool(name="sbm", bufs=3) as sbm, \
                 tc.tile_pool(name="ps_acc", bufs=2, space="PSUM") as ps_acc, \
                 tc.tile_pool(name="ps_tr", bufs=2, space="PSUM") as ps_tr, \
                 tc.tile_pool(name="ps_o", bufs=1, space="PSUM") as ps_o:
                for b in range(NB):
                    e0 = b * T * P
                    xe_b = sb.tile([P, T * DIM], F16, tag="xe")
                    nc.sync.dma_start(
                        xe_b[:].rearrange("p (t d) -> p t d", t=T),
                        din["xe"][e0:e0 + T * P, :].rearrange("(t p) d -> p t d", p=P))
                    xeT_b = sb.tile([P, T * P], F16, tag="xeT")
                    nc.sync.dma_start(xeT_b[:], din["xeT"][:, e0:e0 + T * P])
                    mt_b = sb.tile([P, T * P], F16, tag="mt")
                    nc.sync.dma_start(mt_b[:], din["mtb"][:, e0:e0 + T * P])
                    xTo_t = sbm.tile([P, P], F32, tag="xTo")
                    nc.sync.dma_start(xTo_t[:], din["xTo"][:, b * P:(b + 1) * P])

                    ad1_ps = ps_tr.tile([P, HEADS], F32, tag="asad", padded_shape=[P, 512])
                    nc.tensor.matmul(ad1_ps[:], lhsT=xTo_t[:], rhs=wadT1[:],
                                     start=True, stop=True)
                    ad1_s = sbm.tile([P, HEADS], F16, tag="ad1s")
                    nc.vector.tensor_copy(ad1_s[:], ad1_ps[:])

                    # batched as+ad for all T tiles: [e, 4*T]
                    asad_ps = ps_tr.tile([P, 4 * T], F32, tag="asad", padded_shape=[P, 512])
                    nc.tensor.matmul(asad_ps[:], lhsT=zl_s[:], rhs=zr_s[:, 0:4 * T],
                                     start=True, stop=False)
                    for j in range(T):
                        nc.tensor.matmul(asad_ps[:, 4 * j:4 * j + 4],
                                         lhsT=xeT_b[:, j * P:(j + 1) * P],
                                         rhs=wasT1[:], start=False, stop=False)
                        nc.tensor.matmul(asad_ps[:, 4 * j:4 * j + 4],
                                         lhsT=mt_b[:, j * P:(j + 1) * P],
                                         rhs=ad1_s[:], start=False,
                                         stop=(j == T - 1))
                    t02 = sbm.tile([P, 4 * T], F32, tag="t02")
                    nc.vector.tensor_scalar(out=t02[:], in0=asad_ps[:], scalar1=SLOPE,
                                            scalar2=None, op0=OP.mult)
                    e_s = sbm.tile([P, 4 * T], F32, tag="e")
                    nc.vector.tensor_tensor(out=e_s[:], in0=asad_ps[:], in1=t02[:],
                                            op=OP.max)
                    al_s = sbm.tile([P, 4 * T], F32, tag="al")
                    nc.scalar.activation(al_s[:], e_s[:], AF.Exp)

                    agg01 = ps_acc.tile([P, 258], F32, tag="agg01", padded_shape=[P, 512])
                    agg23 = ps_acc.tile([P, 258], F32, tag="agg23", padded_shape=[P, 512])
                    nc.tensor.matmul(agg01[:], lhsT=zl_s[:], rhs=zr_s[:, 0:258],
                                     start=True, stop=False)
                    nc.tensor.matmul(agg23[:], lhsT=zl_s[:], rhs=zr_s[:, 0:258],
                                     start=True, stop=False)
                    for j in range(T):
                        m_s = sbm.tile([P, P], F16, tag="m")
                        nc.vector.tensor_tensor(
                            out=m_s[:],
                            in0=dstloc[:, b * T + j:b * T + j + 1].to_broadcast([P, P]),
                            in1=iota_f[:], op=OP.is_equal)
                        xw01 = sbm.tile([P, 258], F16, tag="xw01")
                        xw23 = sbm.tile([P, 258], F16, tag="xw23")
                        for k in range(4):
                            dstt = (xw01, xw23)[k // 2]
                            nc.vector.tensor_scalar(
                                out=dstt[:, (k % 2) * P:(k % 2) * P + P],
                                in0=xe_b[:, j * DIM:(j + 1) * DIM],
                                scalar1=al_s[:, 4 * j + k:4 * j + k + 1],
                                scalar2=None, op0=OP.mult)
                        nc.vector.tensor_copy(xw01[:, 256:258],
                                              al_s[:, 4 * j:4 * j + 2])
                        nc.vector.tensor_copy(xw23[:, 256:258],
                                              al_s[:, 4 * j + 2:4 * j + 4])
                        last = (j == T - 1)
                        nc.tensor.matmul(agg01[:], lhsT=m_s[:], rhs=xw01[:],
                                         start=False, stop=last)
                        nc.tensor.matmul(agg23[:], lhsT=m_s[:], rhs=xw23[:],
                                         start=False, stop=last)

                    # block tail
                    denr = sbm.tile([P, HEADS], F32, tag="denr")
                    nc.vector.tensor_scalar(out=denr[:, 0:2], in0=agg01[:, 256:258],
                                            scalar1=1e-16, scalar2=None, op0=OP.add)
                    nc.vector.tensor_scalar(out=denr[:, 2:4], in0=agg23[:, 256:258],
                                            scalar1=1e-16, scalar2=None, op0=OP.add)
                    nc.vector.reciprocal(denr[:], denr[:])
                    aggn = sb.tile([P, 4 * P], F16, tag="aggn")
                    for k in range(4):
                        srct = (agg01, agg23)[k // 2]
                        nc.vector.tensor_scalar(
                            out=aggn[:, k * P:(k + 1) * P],
                            in0=srct[:, (k % 2) * P:(k % 2) * P + P],
                            scalar1=denr[:, k:k + 1], scalar2=None, op0=OP.mult)
                    aggT_ps = ps_o.tile([P, 4 * P], F16, tag="tail", padded_shape=[P, 1024])
                    for k in range(4):
                        nc.tensor.transpose(aggT_ps[:, k * P:(k + 1) * P],
                                            aggn[:, k * P:(k + 1) * P], ident16[:])
                    aggT_s = sb.tile([P, 4 * P], F16, tag="aggTs")
                    nc.vector.tensor_copy(aggT_s[:], aggT_ps[:])

                    o1_ps = ps_o.tile([P, 8 * P], F32, tag="tail", padded_shape=[P, 1024])
                    for c in range(8):
                        nc.tensor.matmul(
                            o1_ps[:, c * P:(c + 1) * P],
                            lhsT=W1T[:, c * P:(c + 1) * P],
                            rhs=aggT_s[:, (c // 2) * P:(c // 2 + 1) * P],
                            start=True, stop=True)
                    h1T = sb.tile([P, 8 * P], F16, tag="h1T")
                    for c in range(8):
                        tt = sbm.tile([P, P], F32, tag="elu_t")
                        rr = sbm.tile([P, P], F32, tag="elu_r")
                        nc.scalar.activation(tt[:], o1_ps[:, c * P:(c + 1) * P],
                                             AF.Exp, bias=b1T[:, c:c + 1])
                        nc.scalar.activation(rr[:], o1_ps[:, c * P:(c + 1) * P],
                                             AF.Relu, bias=b1T[:, c:c + 1])
                        nc.vector.tensor_scalar(out=tt[:], in0=tt[:], scalar1=-1.0,
                                                scalar2=None, op0=OP.add)
                        nc.vector.tensor_tensor(out=h1T[:, c * P:(c + 1) * P],
                                                in0=tt[:], in1=rr[:], op=OP.min)
                    gaug_ps = ps_o.tile([P, 130], F32, tag="tail", padded_shape=[P, 1024])
                    for c in range(8):
                        nc.tensor.matmul(gaug_ps[:], lhsT=h1T[:, c * P:(c + 1) * P],
                                         rhs=W2augP[:, c * 130:(c + 1) * 130],
                                         start=(c == 0), stop=(c == 7))
                    g_sb = sb.tile([P, GW], F16, tag="gsb")
                    nc.vector.tensor_copy(g_sb[:, 0:128], gaug_ps[:, 0:128])
                    nc.vector.memset(g_sb[:, 128:129], 1.0)
                    nc.vector.tensor_copy(g_sb[:, 129:131], gaug_ps[:, 128:130])
                    nc.vector.tensor_copy(ad2_sb[:, b:b + 1], gaug_ps[:, 129:130])
                    nc.sync.dma_start(g_own[b * P:(b + 1) * P, :], g_sb[:])

            # ---------------- AllGather g ----------------
            nc.gpsimd.collective_compute(
                "AllGather", mybir.AluOpType.bypass,
                replica_groups=[list(range(NC))],
                ins=[g_own.ap().opt()], outs=[g_all.ap().opt()])

            # ---------------- L2 ----------------
            ad2_h = cp.tile([P, NB], F16)
            nc.vector.tensor_copy(ad2_h[:], ad2_sb[:])
            with tc.tile_pool(name="sb2", bufs=2) as sb, \
                 tc.tile_pool(name="sbg", bufs=2 * T + 2) as sbg, \
                 tc.tile_pool(name="sbm2", bufs=3) as sbm, \
                 tc.tile_pool(name="ps2_acc", bufs=2, space="PSUM") as ps_acc, \
                 tc.tile_pool(name="ps2_tr", bufs=2, space="PSUM") as ps_tr:
                for b in range(NB):
                    mt_b = sb.tile([P, T * P], F16, tag="mt2")
                    e0 = b * T * P
                    nc.sync.dma_start(mt_b[:], din["mtb"][:, e0:e0 + T * P])
                    g_ts = []
                    as2_all = sbm.tile([P, T], F32, tag="as2a")
                    e2_ps = ps_tr.tile([P, T], F32, tag="e2", padded_shape=[P, 512])
                    nc.tensor.matmul(e2_ps[:], lhsT=zl_s[:], rhs=zr_s[:, 0:T],
                                     start=True, stop=False)
                    for j in range(T):
                        g_t = sbg.tile([P, GW], F16, tag="gt")
                        nc.gpsimd.indirect_dma_start(
                            out=g_t[:], out_offset=None, in_=g_all[:, :],
                            in_offset=IndirectOffsetOnAxis(
                                ap=srcg[:, b * T + j:b * T + j + 1], axis=0))
                        g_ts.append(g_t)
                        nc.vector.tensor_copy(as2_all[:, j:j + 1], g_t[:, 129:130])
                        nc.tensor.matmul(e2_ps[:, j:j + 1],
                                         lhsT=mt_b[:, j * P:(j + 1) * P],
                                         rhs=ad2_h[:, b:b + 1], start=False,
                                         stop=(j == T - 1))
                    s2_s = sbm.tile([P, T], F32, tag="s2s")
                    nc.vector.tensor_tensor(out=s2_s[:], in0=e2_ps[:], in1=as2_all[:],
                                            op=OP.add)
                    t2_s = sbm.tile([P, T], F32, tag="t2s")
                    nc.vector.tensor_scalar(out=t2_s[:], in0=s2_s[:], scalar1=SLOPE,
                                            scalar2=None, op0=OP.mult)
                    e2m = sbm.tile([P, T], F32, tag="e2m")
                    nc.vector.tensor_tensor(out=e2m[:], in0=s2_s[:], in1=t2_s[:],
                                            op=OP.max)
                    al2 = sbm.tile([P, T], F32, tag="al2")
                    nc.scalar.activation(al2[:], e2m[:], AF.Exp)

                    agg2_ps = ps_acc.tile([P, 129], F32, tag="agg2", padded_shape=[P, 512])
                    nc.tensor.matmul(agg2_ps[:], lhsT=zl_s[:], rhs=zr_s[:, 0:129],
                                     start=True, stop=False)
                    for j in range(T):
                        m_s = sbm.tile([P, P], F16, tag="m2")
                        nc.vector.tensor_tensor(
                            out=m_s[:],
                            in0=dstloc[:, b * T + j:b * T + j + 1].to_broadcast([P, P]),
                            in1=iota_f[:], op=OP.is_equal)
                        rhs2 = sbm.tile([P, 129], F16, tag="rhs2")
                        nc.vector.tensor_scalar(out=rhs2[:], in0=g_ts[j][:, 0:129],
                                                scalar1=al2[:, j:j + 1], scalar2=None,
                                                op0=OP.mult)
                        nc.tensor.matmul(agg2_ps[:], lhsT=m_s[:], rhs=rhs2[:],
                                         start=False, stop=(j == T - 1))
                    den2r = sbm.tile([P, 1], F32, tag="den2r")
                    nc.vector.tensor_scalar(out=den2r[:], in0=agg2_ps[:, 128:129],
                                            scalar1=1e-16, scalar2=None, op0=OP.add)
                    nc.vector.reciprocal(den2r[:], den2r[:])
                    y_s = sb.tile([P, EMB], F32, tag="y")
                    nc.vector.tensor_scalar(out=y_s[:], in0=agg2_ps[:, 0:128],
                                            scalar1=den2r[:, 0:1], scalar2=None,
                                            op0=OP.mult)
                    nc.vector.tensor_tensor(out=y_s[:], in0=y_s[:], in1=b2_bc[:],
                                            op=OP.add)
                    tt = sbm.tile([P, EMB], F32, tag="z_t")
                    rr = sbm.tile([P, EMB], F32, tag="z_r")
                    nc.scalar.activation(tt[:], y_s[:], AF.Exp)
                    nc.scalar.activation(rr[:], y_s[:], AF.Relu)
                    nc.vector.tensor_scalar(out=tt[:], in0=tt[:], scalar1=-1.0,
                                            scalar2=None, op0=OP.add)
                    z_sb = sb.tile([P, EMB], F32, tag="zsb")
                    nc.vector.tensor_tensor(out=z_sb[:], in0=tt[:], in1=rr[:], op=OP.min)
                    z_h = sbm.tile([P, EMB], F16, tag="zh")
                    nc.vector.tensor_copy(z_h[:], z_sb[:])
                    nc.sync.dma_start(z_own[b * P:(b + 1) * P, :], z_h[:])
                    nc.sync.dma_start(z_ext[b * P:(b + 1) * P, :], z_sb[:])
                    zT_ps = ps_tr.tile([P, P], F32, tag="ztail", padded_shape=[P, 512])
                    nc.tensor.transpose(zT_ps[:], z_sb[:], ident[:])
                    zT_s = sbm.tile([P, P], F32, tag="zTs")
                    nc.vector.tensor_copy(zT_s[:], zT_ps[:])
                    lg_ps = ps_tr.tile([P, 4], F32, tag="ztail", padded_shape=[P, 512])
                    nc.tensor.matmul(lg_ps[:], lhsT=zT_s[:], rhs=clsWT[:],
                                     start=True, stop=True)
                    nc.vector.tensor_tensor(out=lg_acc[:, b * 4:(b + 1) * 4],
                                            in0=lg_ps[:], in1=clsb_bc[:], op=OP.add)

            # ---------------- AllGather z, preds ----------------
            nc.gpsimd.collective_compute(
                "AllGather", mybir.AluOpType.bypass,
                replica_groups=[list(range(NC))],
                ins=[z_own.ap().opt()], outs=[z_all.ap().opt()])

            with tc.tile_pool(name="sb3", bufs=4) as sb:
                sums = cp.tile([P, npt], F32)
                for q in range(npt):
                    za = sb.tile([P, EMB], F16, tag="za")
                    nc.gpsimd.indirect_dma_start(
                        out=za[:], out_offset=None, in_=z_all[:, :],
                        in_offset=IndirectOffsetOnAxis(ap=pa_t[:, q:q + 1], axis=0))
                    zb = sb.tile([P, EMB], F16, tag="zb")
                    nc.gpsimd.indirect_dma_start(
                        out=zb[:], out_offset=None, in_=z_all[:, :],
                        in_offset=IndirectOffsetOnAxis(ap=pb_t[:, q:q + 1], axis=0))
                    mm = sb.tile([P, EMB], F16, tag="zm")
                    nc.vector.tensor_tensor(out=mm[:], in0=za[:], in1=zb[:],
                                            op=OP.mult)
                    nc.vector.reduce_sum(out=sums[:, q:q + 1], in_=mm[:],
                                         axis=mybir.AxisListType.X)
                nc.scalar.activation(pr_acc[:], sums[:], AF.Sigmoid)
            nc.sync.dma_start(lg_ext[:, :], lg_acc[:])
            nc.sync.dma_start(pr_ext[:, :], pr_acc[:])
    nc.compile()
    return nc


_LAST_EXEC_NS = None


def _trace_kwargs():
    import os
    if not os.environ.get("GAT_TRACE"):
        return {}
    import sys, types
    try:
        import antenv.axon_hooks  # noqa: F401
    except ImportError:
        import antenv
        mod = types.ModuleType("antenv.axon_hooks")
        mod._hook = None
        mod.set_axon_ntff_profile_hook = lambda h: setattr(mod, "_hook", h)
        mod.get_axon_ntff_profile_hook = lambda: mod._hook
        sys.modules["antenv.axon_hooks"] = mod
        antenv.axon_hooks = mod
        try:
            from trn_agent_boot.trn_boot import _ntff_profile_via_ctypes
            mod.set_axon_ntff_profile_hook(
                _ntff_profile_via_ctypes("/opt/axon/libaxon_pjrt.so"))
        except Exception:
            return {}
    return {"trace": True}


def kernel(**inputs):
    global _LAST_EXEC_NS
    from concourse.bass_utils import run_bass_kernel_spmd
    per_core, t_fix, npt, pp, rowv = _prep(**inputs)
    nc = _build(t_fix, npt)
    res = run_bass_kernel_spmd(nc, per_core, core_ids=list(range(NC)),
                               **_trace_kwargs())
    _LAST_EXEC_NS = res.exec_time_ns
    z_cat = np.concatenate([res.results[c]["z_out"] for c in range(NC)], axis=0)
    lg_cat = np.concatenate(
        [res.results[c]["lg_out"].reshape(P, NB, 4).transpose(1, 0, 2)
         .reshape(BLKP, 4) for c in range(NC)], axis=0)
    z = z_cat[rowv]
    logits = lg_cat[rowv]
    preds = np.concatenate(
        [res.results[c]["pr_out"].T.reshape(npt * P)[:pp] for c in range(NC)])
    return z.astype(np.float32), logits.astype(np.float32), preds.astype(np.float32)
